# revision 32
# baseline (speedup 1.0000x reference)
"""Distributed attention kernel for one TRN2 chip (8 NeuronCores).

Problem: multi-head cross-attention
  B=4, TQ=512, TKV=4096, D=1024, H=8 heads (head_dim=128)

Sharding (data-parallel x tensor-parallel, per the hint):
  core c in 0..7 -> (batch b = c % 4, head-group g = c // 4)
  Each core computes heads [4g, 4g+4) for its batch: Wq/Wk/Wv column
  shards, Wo row shard.  Each core writes its full partial out^T; the
  host sums the (c, c+4) pair during the gather (the unshard step for a
  sum-sharded output), so no on-device collective / rendezvous tail.

Device layout (per core; everything transposed so no on-device
transposes are needed - the host passes x^T and mask^T):
  Q^T[dh, t]  = Wq_g^T x_q^T          (4 head-blocks x 8 k-chunks)
  K^T[dh, T]  = Wk_g^T x_kv^T
  V[T, dh]    = x_kv Wv_g             (from x_kv^T chunks as lhsT)
  S^T[T, t]   = K^T_h(block)^T Q^T_h  per head, 32 T-blocks
  P^T         = exp(S^T/sqrt(128)) * mask^T   (no max-subtraction needed:
                scores are O(1) so exp cannot overflow/underflow)
  U^T[dh, t] += V_h(block)^T P^T      accumulated over T-blocks in PSUM
  den        += ones^T P^T            per-block M=1 matmuls into psum
                row 0 (softmax denominators for all t at once)
  U^T *= 1/(den+tiny)                 approx-reciprocal; rows with an
                all-false mask give U = 0 exactly, matching the
                reference's post-softmax wipe
  out^T[o, t] = Wo_g^T U^T (+ bo on group 0 only), DMA out per pair.

Attention loop is software-pipelined: exp+mask-mult for step ds+2
issue right behind that step's S matmuls, so ACT/DVE run a full step
ahead and the PE's semaphore waits are pre-satisfied (LDWEIGHTS
prefetch hides behind streaming).  Per-head finalize (broadcast,
reciprocal quarters, normalize halves) is dripped one op per step into
the next head so it never blocks an engine FIFO; the last head's
finalize overlaps the first 18 output-projection matmuls.

Matmul inputs are bf16 (PE 4x faster than fp32); PSUM accumulation,
softmax denominators and reciprocal stay fp32.
"""

import sys

if "/opt/trn_rl_repo" not in sys.path:
    sys.path.insert(0, "/opt/trn_rl_repo")

import numpy as np
import ml_dtypes
from contextlib import ExitStack

B, TQ, TKV, D, H = 4, 512, 4096, 1024, 8
HD = D // H            # 128 head dim
NCORES = 8
GH = H // 2            # heads per core = 4
GD = GH * HD           # 512 cols per head-group
P = 128
KC = D // P            # 8 contraction chunks
NTB = TKV // P         # 32 T-blocks
NTC = TKV // 512       # 8 T-chunks (DMA granularity)
NOB = D // P           # 8 output o-blocks
SCALE = float(1.0 / np.sqrt(HD))

_CACHED_NC = None


def _build_nc():
    from concourse import mybir, bacc
    from concourse.tile import TileContext

    bf = mybir.dt.bfloat16
    f32 = mybir.dt.float32
    AF = mybir.ActivationFunctionType
    OP = mybir.AluOpType

    nc = bacc.Bacc("TRN2", target_bir_lowering=False, debug=False,
                   num_devices=NCORES)

    # All inputs are pre-tiled on the host into partition-major layouts
    # so every DMA is 128 contiguous multi-KB descriptors.
    xqT = nc.dram_tensor("xqT", [P, KC, TQ], bf, kind="ExternalInput")
    xkvT = nc.dram_tensor("xkvT", [P, NTC, KC, 512], bf, kind="ExternalInput")
    maskT = nc.dram_tensor("maskT", [P, NTB, TQ], bf, kind="ExternalInput")
    Wq = nc.dram_tensor("Wq", [P, KC, GD], bf, kind="ExternalInput")
    Wk = nc.dram_tensor("Wk", [P, KC, GD], bf, kind="ExternalInput")
    Wv = nc.dram_tensor("Wv", [P, KC, GD], bf, kind="ExternalInput")
    Wo = nc.dram_tensor("Wo", [P, GH, D], bf, kind="ExternalInput")
    bq = nc.dram_tensor("bq", [GD], f32, kind="ExternalInput")
    bk = nc.dram_tensor("bk", [GD], f32, kind="ExternalInput")
    bv = nc.dram_tensor("bv", [GD], f32, kind="ExternalInput")
    bo = nc.dram_tensor("bo", [D], f32, kind="ExternalInput")
    out = nc.dram_tensor("out", [P, NOB, TQ], bf, kind="ExternalOutput")

    with TileContext(nc) as tc:
        with ExitStack() as ctx:
            persist = ctx.enter_context(tc.tile_pool(name="persist", bufs=1))
            kvchunk = ctx.enter_context(tc.tile_pool(name="kvchunk", bufs=3))
            work = ctx.enter_context(tc.tile_pool(name="work", bufs=3))
            outp = ctx.enter_context(tc.tile_pool(name="outp", bufs=2))
            # PSUM budget (8 banks): ppool 2x[P,2,TQ] = 4, upool 2x[P,TQ]
            # = 2, dpool 2x[P,TQ] = 2.
            ppool = ctx.enter_context(
                tc.tile_pool(name="ppool", bufs=2, space="PSUM"))
            upool = ctx.enter_context(
                tc.tile_pool(name="upool", bufs=2, space="PSUM"))
            dpool = ctx.enter_context(
                tc.tile_pool(name="dpool", bufs=2, space="PSUM"))

            # ---- constants / weights / biases -------------------------
            # Wq+xq first (whole tensors: 8KB-per-partition descriptors)
            # so the Q projection starts ~6us in, then Wk/kv0/Wv/kv1;
            # mask/Wo are only needed later.
            wq_sb = persist.tile([P, KC, GD], bf)
            xq_sb = persist.tile([P, KC, TQ], bf)
            for q in range(KC):
                nc.sync.dma_start(wq_sb[:, q:q + 1, :],
                                  Wq.ap()[:, q:q + 1, :])
                nc.sync.dma_start(xq_sb[:, q:q + 1, :],
                                  xqT.ap()[:, q:q + 1, :])

            bq_sb = persist.tile([P, GH], f32)
            bk_sb = persist.tile([P, GH], f32)
            nc.sync.dma_start(bq_sb[:], bq.ap().rearrange("(h p) -> p h", p=P))
            nc.sync.dma_start(bk_sb[:], bk.ap().rearrange("(h p) -> p h", p=P))
            bv_row = persist.tile([1, GD], f32)
            nc.sync.dma_start(bv_row[:], bv.ap().unsqueeze(0))
            bv_rep = persist.tile([P, GD], f32)
            nc.gpsimd.partition_broadcast(bv_rep[:], bv_row[:])

            ones_bf = persist.tile([P, 1], bf)
            nc.vector.memset(ones_bf[:], 1.0)

            wk_sb = persist.tile([P, KC, GD], bf)
            wv_sb = persist.tile([P, KC, GD], bf)
            kv_tiles = {}

            def load_kv_chunk(tcknk):
                t = kvchunk.tile([P, KC, 512], bf, name="xkv_t", tag="xkv")
                nc.sync.dma_start(t[:], xkvT.ap()[:, tcknk, :, :])
                kv_tiles[tcknk] = t

            for q in range(4):
                nc.sync.dma_start(wk_sb[:, 2 * q:2 * q + 2, :],
                                  Wk.ap()[:, 2 * q:2 * q + 2, :])
            load_kv_chunk(0)
            for q in range(4):
                nc.sync.dma_start(wv_sb[:, 2 * q:2 * q + 2, :],
                                  Wv.ap()[:, 2 * q:2 * q + 2, :])
            load_kv_chunk(1)

            # ---- Q^T = Wq_g^T x_q^T  (+bq) ----------------------------
            qt_sb = persist.tile([P, GH, TQ], bf)
            for db in range(GH):
                ps = ppool.tile([P, 2, TQ], f32, name="proj_ps",
                                tag="big")[:, 0, :]
                for kc in range(KC):
                    nc.tensor.matmul(ps[:], wq_sb[:, kc, db * P:(db + 1) * P],
                                     xq_sb[:, kc, :],
                                     start=(kc == 0), stop=(kc == KC - 1))
                nc.vector.tensor_tensor(
                    qt_sb[:, db, :], ps[:],
                    bq_sb[:, db:db + 1].to_broadcast([P, TQ]), OP.add)

            # ---- K^T and V over T-chunks ------------------------------
            kt_sb = persist.tile([P, GH, TKV], bf)
            v_sb = persist.tile([P, NTB, GD], bf)
            mask_sb = persist.tile([P, NTB, TQ], bf)
            bo_sb = persist.tile([P, NOB], f32)
            wo_sb = persist.tile([P, GH, D], bf)
            for tcknk in range(NTC):
                if tcknk + 2 < NTC:
                    load_kv_chunk(tcknk + 2)
                xkv_t = kv_tiles.pop(tcknk)
                if tcknk == 1:
                    # queue the bulk "later-phase" loads behind chunks 0-1
                    nc.sync.dma_start(mask_sb[:], maskT.ap())
                    nc.sync.dma_start(wo_sb[:], Wo.ap())
                    nc.sync.dma_start(
                        bo_sb[:], bo.ap().rearrange("(ob p) -> p ob", p=P))
                for db in range(GH):
                    ps = ppool.tile([P, 2, TQ], f32, name="proj_ps",
                                    tag="big")[:, 0, :]
                    for kc in range(KC):
                        nc.tensor.matmul(ps[:], wk_sb[:, kc, db * P:(db + 1) * P],
                                         xkv_t[:, kc, :],
                                         start=(kc == 0), stop=(kc == KC - 1))
                    nc.vector.tensor_tensor(
                        kt_sb[:, db, tcknk * 512:(tcknk + 1) * 512], ps[:],
                        bk_sb[:, db:db + 1].to_broadcast([P, 512]), OP.add)
                for tb in range(4):
                    ps = ppool.tile([P, 2, TQ], f32, name="proj_ps",
                                    tag="big")[:, 0, :]
                    for kc in range(KC):
                        nc.tensor.matmul(ps[:],
                                         xkv_t[:, kc, tb * P:(tb + 1) * P],
                                         wv_sb[:, kc, :],
                                         start=(kc == 0), stop=(kc == KC - 1))
                    nc.vector.tensor_tensor(
                        v_sb[:, tcknk * 4 + tb, :], ps[:], bv_rep[:], OP.add)

            # ---- attention, software-pipelined double-step loop -------
            # Two T-blocks per step: two S-matmuls fill the two banks of
            # one [P, 2, TQ] psum tile, then ONE wide exp + mask-mult.
            # exp/mult run one step AHEAD of the U matmuls that consume
            # them; S prefetch runs two ahead.  Tensor order per step is
            # U (deps long ready), den ones-matmuls, then the next S
            # pair.  With p_t ready a full step early the PE's LDWEIGHTS
            # prefetch is never semaphore-blocked.
            ut_sb = persist.tile([P, GH, TQ], bf)
            NDS = GH * NTB // 2
            s_tiles, p_tiles = {}, {}
            u_tiles = [None] * GH
            den_tiles = [None] * GH
            fin_pend = None

            def s2_mm(ds):
                t2 = ppool.tile([P, 2, TQ], f32, name="s2_ps", tag="big")
                for k in range(2):
                    h, j = divmod(ds * 2 + k, NTB)
                    nc.tensor.matmul(t2[:, k, :],
                                     kt_sb[:, h, j * P:(j + 1) * P],
                                     qt_sb[:, h, :], start=True, stop=True)
                return t2

            def exp_mult(ds):
                h, j0 = divmod(ds * 2, NTB)
                t2 = s_tiles.pop(ds)
                praw = work.tile([P, 2, TQ], bf, tag="praw", bufs=3)
                nc.scalar.activation(praw[:], t2[:], AF.Exp, scale=SCALE)
                p_t = work.tile([P, 2, TQ], bf, tag="p_t", bufs=4)
                nc.vector.tensor_tensor(p_t[:], praw[:],
                                        mask_sb[:, j0:j0 + 2, :], OP.mult)
                p_tiles[ds] = p_t

            def fin_chain(h, drow):
                # 1/den broadcast + normalize.  Returns a list of small
                # thunks so the DVE work can be dripped one op per step
                # (a single fat op at a head boundary blocks the DVE
                # FIFO and stalls the U matmuls behind it).
                rep = work.tile([P, TQ], f32, tag="rep", bufs=2)
                rcp = work.tile([P, TQ], f32, tag="rcp", bufs=2)
                u_ps = u_tiles[h]
                ops = [lambda: nc.gpsimd.partition_broadcast(rep[:], drow[:])]
                for q in range(4):
                    sl = slice(q * TQ // 4, (q + 1) * TQ // 4)
                    ops.append(lambda sl=sl: nc.vector.reciprocal_approx_fast(
                        rcp[:, sl], rep[:, sl]))
                for g in range(2):
                    sl = slice(g * TQ // 2, (g + 1) * TQ // 2)
                    ops.append(lambda sl=sl: nc.vector.tensor_tensor(
                        ut_sb[:, h, sl], u_ps[:, sl], rcp[:, sl], OP.mult))
                return ops

            # prologue
            s_tiles[0] = s2_mm(0)
            s_tiles[1] = s2_mm(1)
            exp_mult(0)
            exp_mult(1)
            fin_ops = []

            for ds in range(NDS):
                h, j0 = divmod(ds * 2, NTB)
                if j0 == 0:
                    u_tiles[h] = upool.tile([P, TQ], f32, name="u_ps",
                                            tag="u_ps")
                    den_tiles[h] = dpool.tile([P, TQ], f32, name="den_ps",
                                              tag="den_ps")
                if fin_ops:
                    fin_ops.pop(0)()  # drip one finalize op per step
                p_t = p_tiles.pop(ds)
                # Grouped same-type pairs (U,U,den,den,S,S) measure
                # ~150ns/step faster than any interleaving of U/den/S.
                for k in range(2):
                    j = j0 + k
                    nc.tensor.matmul(u_tiles[h][:],
                                     v_sb[:, j, h * P:(h + 1) * P],
                                     p_t[:, k, :],
                                     start=(j == 0), stop=(j == NTB - 1))
                for k in range(2):
                    j = j0 + k
                    nc.tensor.matmul(den_tiles[h][0:1, :], ones_bf[:],
                                     p_t[:, k, :],
                                     start=(j == 0), stop=(j == NTB - 1))
                if ds + 2 < NDS:
                    s_tiles[ds + 2] = s2_mm(ds + 2)
                    # exp/mask-mult two steps ahead, right behind their
                    # S matmuls: ACT and DVE get a full step of slack
                    exp_mult(ds + 2)
                if j0 == NTB - 2:
                    # den row -> sbuf on the idle ACT engine; +1e-30 is
                    # the all-masked-row guard, folded in for free
                    drow = work.tile([1, TQ], f32, tag="drow", bufs=2)
                    nc.scalar.activation(drow[:], den_tiles[h][0:1, :],
                                         AF.Copy, bias=1e-30)
                    fin_pend = (h, drow)
                if fin_pend is not None and j0 == 0 and ds > 0:
                    fin_ops = fin_chain(*fin_pend)
                    fin_pend = None

            # ---- out^T = Wo_g^T U^T (+bo), direct DMA out -------------
            # Partial (head-group) output; the host sums the pair.
            # Phase 1 accumulates heads 0..2 for 6 o-blocks across every
            # free psum bank while head 3's finalize chain runs on
            # gpsimd/DVE; phase 2 adds head 3 and the last 2 o-blocks.
            o_halves = [outp.tile([P, NOB // 2, TQ], bf, name="o_half",
                                  tag="o_half") for _ in range(2)]
            ppA = ppool.tile([P, 2, TQ], f32, name="o_ps", tag="big")
            ppB = ppool.tile([P, 2, TQ], f32, name="o_ps", tag="big")
            ogrp = [ppA[:, 0, :], ppA[:, 1, :], ppB[:, 0, :], ppB[:, 1, :],
                    upool.tile([P, TQ], f32, name="o_ps", tag="u_ps")[:],
                    dpool.tile([P, TQ], f32, name="o_ps", tag="den_ps")[:]]
            for oi in range(6):
                for hc in range(GH - 1):
                    nc.tensor.matmul(ogrp[oi],
                                     wo_sb[:, hc, oi * P:(oi + 1) * P],
                                     ut_sb[:, hc, :],
                                     start=(hc == 0), stop=False)
            # head 3 finalize; overlaps the phase-1 matmuls above
            for op in fin_chain(*fin_pend):
                op()
            fin_pend = None

            def o_emit(oi, ps):
                # alternate the psum->sbuf bias-add between DVE and ACT
                # (bo is per-partition in the out^T layout) so the tail
                # adds don't serialize on one engine
                if oi % 2 == 0:
                    nc.vector.tensor_tensor(
                        o_halves[oi // 4][:, oi % 4, :], ps,
                        bo_sb[:, oi:oi + 1].to_broadcast([P, TQ]), OP.add)
                else:
                    nc.scalar.add(o_halves[oi // 4][:, oi % 4, :], ps,
                                  bo_sb[:, oi:oi + 1])

            for oi in range(6):
                nc.tensor.matmul(ogrp[oi],
                                 wo_sb[:, GH - 1, oi * P:(oi + 1) * P],
                                 ut_sb[:, GH - 1, :], start=False, stop=True)
                o_emit(oi, ogrp[oi])
            for oi in (6, 7):
                ps = ppool.tile([P, 2, TQ], f32, name="o_ps",
                                tag="big")[:, 0, :]
                for hc in range(GH):
                    nc.tensor.matmul(ps[:],
                                     wo_sb[:, hc, oi * P:(oi + 1) * P],
                                     ut_sb[:, hc, :],
                                     start=(hc == 0), stop=(hc == GH - 1))
                o_emit(oi, ps[:])
            for pair in range(4):
                nc.sync.dma_start(
                    out.ap()[:, 2 * pair:2 * pair + 2, :],
                    o_halves[pair // 2][:, 2 * (pair % 2):2 * (pair % 2) + 2, :])

    nc.finalize()
    return nc


def _shard_inputs(inputs_q, inputs_kv, attention_mask, Wq, bq, Wk, bk, Wv, bv,
                  Wo, bo):
    bf16 = ml_dtypes.bfloat16
    f32 = np.float32

    def ptile(a2d, inner):
        """[R, C] row-major -> [P, R//P, C] partition-major, contiguous."""
        r, c = a2d.shape
        return np.ascontiguousarray(
            a2d.reshape(r // P, P, c).transpose(1, 0, 2)).astype(inner)

    in_maps = []
    xqT = [ptile(inputs_q[b].T, bf16) for b in range(B)]          # [P,KC,TQ]
    xkvT = [ptile(inputs_kv[b].T, bf16)                           # [P,NTC,KC,512]
            .reshape(P, KC, NTC, 512).transpose(0, 2, 1, 3).copy()
            for b in range(B)]
    maskT = [ptile(attention_mask[b].T.astype(np.float32), bf16)  # [P,NTB,TQ]
             for b in range(B)]
    for c in range(NCORES):
        b, g = c % B, c // B  # pair = (b, b+4)
        sl = slice(g * GD, (g + 1) * GD)
        in_maps.append({
            "xqT": xqT[b],
            "xkvT": xkvT[b],
            "maskT": maskT[b],
            "Wq": ptile(np.ascontiguousarray(Wq[:, sl]), bf16),
            "Wk": ptile(np.ascontiguousarray(Wk[:, sl]), bf16),
            "Wv": ptile(np.ascontiguousarray(Wv[:, sl]), bf16),
            "Wo": ptile(np.ascontiguousarray(Wo[sl, :]), bf16),
            "bq": np.ascontiguousarray(bq[sl]).astype(f32),
            "bk": np.ascontiguousarray(bk[sl]).astype(f32),
            "bv": np.ascontiguousarray(bv[sl]).astype(f32),
            "bo": (bo.astype(f32) if g == 0 else np.zeros(D, f32)),
        })
    return in_maps


def kernel(_trace=False, **inputs):
    global _CACHED_NC
    from concourse import bass_utils

    arrs = {k: np.asarray(v) for k, v in inputs.items()}
    in_maps = _shard_inputs(**arrs)

    if _CACHED_NC is None:
        _CACHED_NC = _build_nc()

    res = bass_utils.run_bass_kernel_spmd(
        _CACHED_NC, in_maps, core_ids=list(range(NCORES)), trace=_trace)

    full = np.empty((B, TQ, D), np.float32)
    for b in range(B):
        # pair (b, b+4): sum the two head-group partial outputs on the
        # host (the unshard step for a sum-sharded output)
        outT = (res.results[b]["out"].astype(np.float32)
                + res.results[b + 4]["out"].astype(np.float32))
        full[b] = outT.transpose(1, 0, 2).reshape(D, TQ).T
    if _trace:
        return full, res
    return full


# revision 34
# speedup vs baseline: 1.1878x; 1.1878x over previous
"""Distributed attention kernel for one TRN2 chip (8 NeuronCores).

Problem: multi-head cross-attention
  B=4, TQ=512, TKV=4096, D=1024, H=8 heads (head_dim=128)

Sharding (data-parallel x tensor-parallel, per the hint):
  core c in 0..7 -> (batch b = c % 4, head-group g = c // 4)
  Each core computes heads [4g, 4g+4) for its batch: Wq/Wk/Wv column
  shards, Wo row shard.  Each core writes its full partial out^T; the
  host sums the (c, c+4) pair during the gather (the unshard step for a
  sum-sharded output), so no on-device collective / rendezvous tail.

Device layout (per core; everything transposed so no on-device
transposes are needed - the host passes x^T and mask^T):
  Q^T[dh, t]  = Wq_g^T x_q^T          (4 head-blocks x 8 k-chunks)
  K^T[dh, T]  = Wk_g^T x_kv^T
  V[T, dh]    = x_kv Wv_g             (from x_kv^T chunks as lhsT)
  S^T[T, t]   = K^T_h(block)^T Q^T_h  per head, 32 T-blocks
  P^T         = exp(S^T/sqrt(128)) * mask^T   (no max-subtraction needed:
                scores are O(1) so exp cannot overflow/underflow)
  U^T[dh, t] += V_h(block)^T P^T      accumulated over T-blocks in PSUM
  den        += ones^T P^T            per-block M=1 matmuls into psum
                row 0 (softmax denominators for all t at once)
  U^T *= 1/(den+tiny)                 approx-reciprocal; rows with an
                all-false mask give U = 0 exactly, matching the
                reference's post-softmax wipe
  out^T[o, t] = Wo_g^T U^T (+ bo on group 0 only), DMA out per pair.

Attention loop is software-pipelined: exp+mask-mult for step ds+2
issue right behind that step's S matmuls, so ACT/DVE run a full step
ahead and the PE's semaphore waits are pre-satisfied (LDWEIGHTS
prefetch hides behind streaming).  Per-head finalize (broadcast,
reciprocal quarters, normalize halves) is dripped one op per step into
the next head so it never blocks an engine FIFO; the last head's
finalize overlaps the first 18 output-projection matmuls.

Matmul inputs are bf16 (PE 4x faster than fp32); PSUM accumulation,
softmax denominators and reciprocal stay fp32.
"""

import sys

if "/opt/trn_rl_repo" not in sys.path:
    sys.path.insert(0, "/opt/trn_rl_repo")

import numpy as np
import ml_dtypes
from contextlib import ExitStack

B, TQ, TKV, D, H = 4, 512, 4096, 1024, 8
HD = D // H            # 128 head dim
NCORES = 8
GH = H // 2            # heads per core = 4
GD = GH * HD           # 512 cols per head-group
P = 128
KC = D // P            # 8 contraction chunks
NTB = TKV // P         # 32 T-blocks
NTC = TKV // 512       # 8 T-chunks (DMA granularity)
NOB = D // P           # 8 output o-blocks
SCALE = float(1.0 / np.sqrt(HD))

_CACHED_NC = None


def _build_nc():
    from concourse import mybir, bacc
    from concourse.tile import TileContext

    bf = mybir.dt.bfloat16
    f32 = mybir.dt.float32
    AF = mybir.ActivationFunctionType
    OP = mybir.AluOpType

    nc = bacc.Bacc("TRN2", target_bir_lowering=False, debug=False,
                   num_devices=NCORES)

    # All inputs are pre-tiled on the host into partition-major layouts
    # so every DMA is 128 contiguous multi-KB descriptors.
    xqT = nc.dram_tensor("xqT", [P, KC, TQ], bf, kind="ExternalInput")
    xkvT = nc.dram_tensor("xkvT", [P, NTC, KC, 512], bf, kind="ExternalInput")
    maskT = nc.dram_tensor("maskT", [P, NTB, TQ], bf, kind="ExternalInput")
    Wq = nc.dram_tensor("Wq", [P, KC, GD], bf, kind="ExternalInput")
    Wk = nc.dram_tensor("Wk", [P, KC, GD], bf, kind="ExternalInput")
    Wv = nc.dram_tensor("Wv", [P, KC, GD], bf, kind="ExternalInput")
    Wo = nc.dram_tensor("Wo", [P, GH, D], bf, kind="ExternalInput")
    bq = nc.dram_tensor("bq", [GD], f32, kind="ExternalInput")
    bk = nc.dram_tensor("bk", [GD], f32, kind="ExternalInput")
    bv = nc.dram_tensor("bv", [GD], f32, kind="ExternalInput")
    bo = nc.dram_tensor("bo", [D], f32, kind="ExternalInput")
    out = nc.dram_tensor("out", [P, NOB, TQ], bf, kind="ExternalOutput")

    with TileContext(nc) as tc:
        with ExitStack() as ctx:
            persist = ctx.enter_context(tc.tile_pool(name="persist", bufs=1))
            kvchunk = ctx.enter_context(tc.tile_pool(name="kvchunk", bufs=3))
            work = ctx.enter_context(tc.tile_pool(name="work", bufs=3))
            outp = ctx.enter_context(tc.tile_pool(name="outp", bufs=2))
            # PSUM budget (8 banks): ppool 2x[P,2,TQ] = 4, upool 2x[P,TQ]
            # = 2, dpool 2x[P,TQ] = 2.
            ppool = ctx.enter_context(
                tc.tile_pool(name="ppool", bufs=2, space="PSUM"))
            upool = ctx.enter_context(
                tc.tile_pool(name="upool", bufs=2, space="PSUM"))
            dpool = ctx.enter_context(
                tc.tile_pool(name="dpool", bufs=2, space="PSUM"))

            # ---- constants / weights / biases -------------------------
            # Wq+xq first (whole tensors: 8KB-per-partition descriptors)
            # so the Q projection starts ~6us in, then Wk/kv0/Wv/kv1;
            # mask/Wo are only needed later.
            wq_sb = persist.tile([P, KC, GD], bf)
            xq_sb = persist.tile([P, KC, TQ], bf)
            for q in range(KC):
                nc.sync.dma_start(wq_sb[:, q:q + 1, :],
                                  Wq.ap()[:, q:q + 1, :])
                nc.sync.dma_start(xq_sb[:, q:q + 1, :],
                                  xqT.ap()[:, q:q + 1, :])

            bq_sb = persist.tile([P, GH], f32)
            bk_sb = persist.tile([P, GH], f32)
            nc.sync.dma_start(bq_sb[:], bq.ap().rearrange("(h p) -> p h", p=P))
            nc.sync.dma_start(bk_sb[:], bk.ap().rearrange("(h p) -> p h", p=P))
            bv_row = persist.tile([1, GD], f32)
            nc.sync.dma_start(bv_row[:], bv.ap().unsqueeze(0))
            bv_rep = persist.tile([P, GD], f32)
            nc.gpsimd.partition_broadcast(bv_rep[:], bv_row[:])

            ones_bf = persist.tile([P, 1], bf)
            nc.vector.memset(ones_bf[:], 1.0)

            wk_sb = persist.tile([P, KC, GD], bf)
            wv_sb = persist.tile([P, KC, GD], bf)
            kv_tiles = {}

            def load_kv_chunk(tcknk):
                t = kvchunk.tile([P, KC, 512], bf, name="xkv_t", tag="xkv")
                nc.sync.dma_start(t[:], xkvT.ap()[:, tcknk, :, :])
                kv_tiles[tcknk] = t

            for q in range(4):
                nc.sync.dma_start(wk_sb[:, 2 * q:2 * q + 2, :],
                                  Wk.ap()[:, 2 * q:2 * q + 2, :])
            load_kv_chunk(0)
            for q in range(4):
                nc.sync.dma_start(wv_sb[:, 2 * q:2 * q + 2, :],
                                  Wv.ap()[:, 2 * q:2 * q + 2, :])
            load_kv_chunk(1)

            # ---- Q^T = Wq_g^T x_q^T  (+bq) ----------------------------
            qt_sb = persist.tile([P, GH, TQ], bf)
            for db in range(GH):
                ps = ppool.tile([P, 2, TQ], f32, name="proj_ps",
                                tag="big")[:, 0, :]
                for kc in range(KC):
                    nc.tensor.matmul(ps[:], wq_sb[:, kc, db * P:(db + 1) * P],
                                     xq_sb[:, kc, :],
                                     start=(kc == 0), stop=(kc == KC - 1))
                nc.vector.tensor_tensor(
                    qt_sb[:, db, :], ps[:],
                    bq_sb[:, db:db + 1].to_broadcast([P, TQ]), OP.add)

            # ---- K^T and V over T-chunks ------------------------------
            kt_sb = persist.tile([P, GH, TKV], bf)
            v_sb = persist.tile([P, NTB, GD], bf)
            mask_sb = persist.tile([P, NTB, TQ], bf)
            bo_sb = persist.tile([P, NOB], f32)
            wo_sb = persist.tile([P, GH, D], bf)
            for tcknk in range(NTC):
                if tcknk + 2 < NTC:
                    load_kv_chunk(tcknk + 2)
                xkv_t = kv_tiles.pop(tcknk)
                if tcknk == 1:
                    # queue the bulk "later-phase" loads behind chunks 0-1
                    nc.sync.dma_start(mask_sb[:], maskT.ap())
                    nc.sync.dma_start(wo_sb[:], Wo.ap())
                    nc.sync.dma_start(
                        bo_sb[:], bo.ap().rearrange("(ob p) -> p ob", p=P))
                for db in range(GH):
                    ps = ppool.tile([P, 2, TQ], f32, name="proj_ps",
                                    tag="big")[:, 0, :]
                    for kc in range(KC):
                        nc.tensor.matmul(ps[:], wk_sb[:, kc, db * P:(db + 1) * P],
                                         xkv_t[:, kc, :],
                                         start=(kc == 0), stop=(kc == KC - 1))
                    nc.vector.tensor_tensor(
                        kt_sb[:, db, tcknk * 512:(tcknk + 1) * 512], ps[:],
                        bk_sb[:, db:db + 1].to_broadcast([P, 512]), OP.add)
                for tb in range(4):
                    ps = ppool.tile([P, 2, TQ], f32, name="proj_ps",
                                    tag="big")[:, 0, :]
                    for kc in range(KC):
                        nc.tensor.matmul(ps[:],
                                         xkv_t[:, kc, tb * P:(tb + 1) * P],
                                         wv_sb[:, kc, :],
                                         start=(kc == 0), stop=(kc == KC - 1))
                    nc.vector.tensor_tensor(
                        v_sb[:, tcknk * 4 + tb, :], ps[:], bv_rep[:], OP.add)

            # ---- attention, software-pipelined double-step loop -------
            # Two T-blocks per step: two S-matmuls fill the two banks of
            # one [P, 2, TQ] psum tile, then ONE wide exp + mask-mult.
            # exp/mult run one step AHEAD of the U matmuls that consume
            # them; S prefetch runs two ahead.  Tensor order per step is
            # U (deps long ready), den ones-matmuls, then the next S
            # pair.  With p_t ready a full step early the PE's LDWEIGHTS
            # prefetch is never semaphore-blocked.
            ut_sb = persist.tile([P, GH, TQ], bf)
            NDS = GH * NTB // 2
            s_tiles, p_tiles = {}, {}
            u_tiles = [None] * GH
            den_tiles = [None] * GH
            fin_pend = None

            def s2_mm(ds):
                t2 = ppool.tile([P, 2, TQ], f32, name="s2_ps", tag="big")
                for k in range(2):
                    h, j = divmod(ds * 2 + k, NTB)
                    nc.tensor.matmul(t2[:, k, :],
                                     kt_sb[:, h, j * P:(j + 1) * P],
                                     qt_sb[:, h, :], start=True, stop=True)
                return t2

            def exp_mult(ds):
                h, j0 = divmod(ds * 2, NTB)
                t2 = s_tiles.pop(ds)
                praw = work.tile([P, 2, TQ], bf, tag="praw", bufs=3)
                nc.scalar.activation(praw[:], t2[:], AF.Exp, scale=SCALE)
                p_t = work.tile([P, 2, TQ], bf, tag="p_t", bufs=4)
                nc.vector.tensor_tensor(p_t[:], praw[:],
                                        mask_sb[:, j0:j0 + 2, :], OP.mult)
                p_tiles[ds] = p_t

            def fin_chain(h, drow):
                # 1/den broadcast + normalize.  Returns a list of small
                # thunks so the DVE work can be dripped one op per step
                # (a single fat op at a head boundary blocks the DVE
                # FIFO and stalls the U matmuls behind it).
                rep = work.tile([P, TQ], f32, tag="rep", bufs=2)
                rcp = work.tile([P, TQ], f32, tag="rcp", bufs=2)
                u_ps = u_tiles[h]
                ops = [lambda: nc.gpsimd.partition_broadcast(rep[:], drow[:])]
                for q in range(4):
                    sl = slice(q * TQ // 4, (q + 1) * TQ // 4)
                    ops.append(lambda sl=sl: nc.vector.reciprocal_approx_fast(
                        rcp[:, sl], rep[:, sl]))
                for g in range(2):
                    sl = slice(g * TQ // 2, (g + 1) * TQ // 2)
                    ops.append(lambda sl=sl: nc.vector.tensor_tensor(
                        ut_sb[:, h, sl], u_ps[:, sl], rcp[:, sl], OP.mult))
                return ops

            # prologue
            s_tiles[0] = s2_mm(0)
            s_tiles[1] = s2_mm(1)
            exp_mult(0)
            exp_mult(1)
            fin_ops = []

            for ds in range(NDS):
                h, j0 = divmod(ds * 2, NTB)
                if j0 == 0:
                    u_tiles[h] = upool.tile([P, TQ], f32, name="u_ps",
                                            tag="u_ps")
                    den_tiles[h] = dpool.tile([P, TQ], f32, name="den_ps",
                                              tag="den_ps")
                if fin_ops:
                    fin_ops.pop(0)()  # drip one finalize op per step
                p_t = p_tiles.pop(ds)
                # Grouped same-type pairs (U,U,den,den,S,S) measure
                # ~150ns/step faster than any interleaving of U/den/S.
                for k in range(2):
                    j = j0 + k
                    nc.tensor.matmul(u_tiles[h][:],
                                     v_sb[:, j, h * P:(h + 1) * P],
                                     p_t[:, k, :],
                                     start=(j == 0), stop=(j == NTB - 1))
                for k in range(2):
                    j = j0 + k
                    nc.tensor.matmul(den_tiles[h][0:1, :], ones_bf[:],
                                     p_t[:, k, :],
                                     start=(j == 0), stop=(j == NTB - 1))
                if ds + 2 < NDS:
                    s_tiles[ds + 2] = s2_mm(ds + 2)
                    # exp/mask-mult two steps ahead, right behind their
                    # S matmuls: ACT and DVE get a full step of slack
                    exp_mult(ds + 2)
                if j0 == NTB - 2:
                    # den row -> sbuf on the idle ACT engine; +1e-30 is
                    # the all-masked-row guard, folded in for free
                    drow = work.tile([1, TQ], f32, tag="drow", bufs=2)
                    nc.scalar.activation(drow[:], den_tiles[h][0:1, :],
                                         AF.Copy, bias=1e-30)
                    fin_pend = (h, drow)
                if fin_pend is not None and j0 == 0 and ds > 0:
                    fin_ops = fin_chain(*fin_pend)
                    fin_pend = None

            # ---- out^T = Wo_g^T U^T (+bo), direct DMA out -------------
            # Partial (head-group) output; the host sums the pair.
            # Phase 1 accumulates heads 0..2 for 6 o-blocks across every
            # free psum bank while head 3's finalize chain runs on
            # gpsimd/DVE; phase 2 adds head 3 and the last 2 o-blocks.
            o_halves = [outp.tile([P, NOB // 2, TQ], bf, name="o_half",
                                  tag="o_half") for _ in range(2)]
            ppA = ppool.tile([P, 2, TQ], f32, name="o_ps", tag="big")
            ppB = ppool.tile([P, 2, TQ], f32, name="o_ps", tag="big")
            ogrp = [ppA[:, 0, :], ppA[:, 1, :], ppB[:, 0, :], ppB[:, 1, :],
                    upool.tile([P, TQ], f32, name="o_ps", tag="u_ps")[:],
                    dpool.tile([P, TQ], f32, name="o_ps", tag="den_ps")[:]]
            for oi in range(6):
                for hc in range(GH - 1):
                    nc.tensor.matmul(ogrp[oi],
                                     wo_sb[:, hc, oi * P:(oi + 1) * P],
                                     ut_sb[:, hc, :],
                                     start=(hc == 0), stop=False)
            # head 3 finalize; overlaps the phase-1 matmuls above
            for op in fin_chain(*fin_pend):
                op()
            fin_pend = None

            def o_emit(oi, ps):
                # alternate the psum->sbuf bias-add between DVE and ACT
                # (bo is per-partition in the out^T layout) so the tail
                # adds don't serialize on one engine
                if oi % 2 == 0:
                    nc.vector.tensor_tensor(
                        o_halves[oi // 4][:, oi % 4, :], ps,
                        bo_sb[:, oi:oi + 1].to_broadcast([P, TQ]), OP.add)
                else:
                    nc.scalar.add(o_halves[oi // 4][:, oi % 4, :], ps,
                                  bo_sb[:, oi:oi + 1])

            for oi in range(6):
                nc.tensor.matmul(ogrp[oi],
                                 wo_sb[:, GH - 1, oi * P:(oi + 1) * P],
                                 ut_sb[:, GH - 1, :], start=False, stop=True)
                o_emit(oi, ogrp[oi])
            for oi in (6, 7):
                ps = ppool.tile([P, 2, TQ], f32, name="o_ps",
                                tag="big")[:, 0, :]
                for hc in range(GH):
                    nc.tensor.matmul(ps[:],
                                     wo_sb[:, hc, oi * P:(oi + 1) * P],
                                     ut_sb[:, hc, :],
                                     start=(hc == 0), stop=(hc == GH - 1))
                o_emit(oi, ps[:])
            for pair in range(4):
                nc.sync.dma_start(
                    out.ap()[:, 2 * pair:2 * pair + 2, :],
                    o_halves[pair // 2][:, 2 * (pair % 2):2 * (pair % 2) + 2, :])

    nc.finalize()
    return nc


def _shard_inputs(inputs_q, inputs_kv, attention_mask, Wq, bq, Wk, bk, Wv, bv,
                  Wo, bo):
    bf16 = ml_dtypes.bfloat16
    f32 = np.float32

    def ptile(a2d, inner):
        """[R, C] row-major -> [P, R//P, C] partition-major, contiguous."""
        r, c = a2d.shape
        return np.ascontiguousarray(
            a2d.reshape(r // P, P, c).transpose(1, 0, 2)).astype(inner)

    in_maps = []
    xqT = [ptile(inputs_q[b].T, bf16) for b in range(B)]          # [P,KC,TQ]
    xkvT = [ptile(inputs_kv[b].T, bf16)                           # [P,NTC,KC,512]
            .reshape(P, KC, NTC, 512).transpose(0, 2, 1, 3).copy()
            for b in range(B)]
    maskT = [ptile(attention_mask[b].T.astype(np.float32), bf16)  # [P,NTB,TQ]
             for b in range(B)]
    for c in range(NCORES):
        b, g = c % B, c // B  # pair = (b, b+4)
        sl = slice(g * GD, (g + 1) * GD)
        in_maps.append({
            "xqT": xqT[b],
            "xkvT": xkvT[b],
            "maskT": maskT[b],
            "Wq": ptile(np.ascontiguousarray(Wq[:, sl]), bf16),
            "Wk": ptile(np.ascontiguousarray(Wk[:, sl]), bf16),
            "Wv": ptile(np.ascontiguousarray(Wv[:, sl]), bf16),
            "Wo": ptile(np.ascontiguousarray(Wo[sl, :]), bf16),
            "bq": np.ascontiguousarray(bq[sl]).astype(f32),
            "bk": np.ascontiguousarray(bk[sl]).astype(f32),
            "bv": np.ascontiguousarray(bv[sl]).astype(f32),
            "bo": (bo.astype(f32) if g == 0 else np.zeros(D, f32)),
        })
    return in_maps


def kernel(_trace=False, **inputs):
    global _CACHED_NC
    from concourse import bass_utils

    arrs = {k: np.asarray(v) for k, v in inputs.items()}
    in_maps = _shard_inputs(**arrs)

    if _CACHED_NC is None:
        _CACHED_NC = _build_nc()

    res = bass_utils.run_bass_kernel_spmd(
        _CACHED_NC, in_maps, core_ids=list(range(NCORES)), trace=_trace)

    full = np.empty((B, TQ, D), np.float32)
    for b in range(B):
        # pair (b, b+4): sum the two head-group partial outputs on the
        # host (the unshard step for a sum-sharded output)
        outT = (res.results[b]["out"].astype(np.float32)
                + res.results[b + 4]["out"].astype(np.float32))
        full[b] = outT.transpose(1, 0, 2).reshape(D, TQ).T
    if _trace:
        return full, res
    return full


# revision 37
# speedup vs baseline: 1.2727x; 1.0714x over previous
"""Distributed attention kernel for one TRN2 chip (8 NeuronCores).

Problem: multi-head cross-attention
  B=4, TQ=512, TKV=4096, D=1024, H=8 heads (head_dim=128)

Sharding (data-parallel x tensor-parallel, per the hint):
  core c in 0..7 -> (batch b = c % 4, head-group g = c // 4)
  Each core computes heads [4g, 4g+4) for its batch: Wq/Wk/Wv column
  shards, Wo row shard.  Each core writes its full partial out^T; the
  host sums the (c, c+4) pair during the gather (the unshard step for a
  sum-sharded output), so no on-device collective / rendezvous tail.

Device layout (per core; everything transposed so no on-device
transposes are needed - the host passes x^T and mask^T):
  Q^T[dh, t]  = Wq_g^T x_q^T          (4 head-blocks x 8 k-chunks)
  K^T[dh, T]  = Wk_g^T x_kv^T
  V[T, dh]    = x_kv Wv_g             (from x_kv^T chunks as lhsT)
  S^T[T, t]   = K^T_h(block)^T Q^T_h  per head, 32 T-blocks
  P^T         = exp(S^T/sqrt(128)) * mask^T   (no max-subtraction needed:
                scores are O(1) so exp cannot overflow/underflow)
  U^T[dh, t] += V_h(block)^T P^T      accumulated over T-blocks in PSUM
  den        += ones^T P^T            per-block M=1 matmuls into psum
                row 0 (softmax denominators for all t at once)
  U^T *= 1/(den+tiny)                 approx-reciprocal; rows with an
                all-false mask give U = 0 exactly, matching the
                reference's post-softmax wipe
  out^T[o, t] = Wo_g^T U^T (+ bo on group 0 only), DMA out per pair.

Attention loop is software-pipelined: exp+mask-mult for step ds+2
issue right behind that step's S matmuls, so ACT/DVE run a full step
ahead and the PE's semaphore waits are pre-satisfied (LDWEIGHTS
prefetch hides behind streaming).  Per-head finalize (broadcast,
reciprocal quarters, normalize halves) is dripped one op per step into
the next head so it never blocks an engine FIFO; the last head's
finalize overlaps the first 18 output-projection matmuls.

Matmul inputs are bf16 (PE 4x faster than fp32); PSUM accumulation,
softmax denominators and reciprocal stay fp32.
"""

import sys

if "/opt/trn_rl_repo" not in sys.path:
    sys.path.insert(0, "/opt/trn_rl_repo")

import numpy as np
import ml_dtypes
from contextlib import ExitStack

B, TQ, TKV, D, H = 4, 512, 4096, 1024, 8
HD = D // H            # 128 head dim
NCORES = 8
GH = H // 2            # heads per core = 4
GD = GH * HD           # 512 cols per head-group
P = 128
KC = D // P            # 8 contraction chunks
NTB = TKV // P         # 32 T-blocks
NTC = TKV // 512       # 8 T-chunks (DMA granularity)
NOB = D // P           # 8 output o-blocks
SCALE = float(1.0 / np.sqrt(HD))

_CACHED_NC = None


def _build_nc():
    from concourse import mybir, bacc
    from concourse.tile import TileContext

    bf = mybir.dt.bfloat16
    f32 = mybir.dt.float32
    AF = mybir.ActivationFunctionType
    OP = mybir.AluOpType

    nc = bacc.Bacc("TRN2", target_bir_lowering=False, debug=False,
                   num_devices=NCORES)

    # All inputs are pre-tiled on the host into partition-major layouts
    # so every DMA is 128 contiguous multi-KB descriptors.
    xqT = nc.dram_tensor("xqT", [P, KC, TQ], bf, kind="ExternalInput")
    xkvT = nc.dram_tensor("xkvT", [P, NTC, KC, 512], bf, kind="ExternalInput")
    maskT = nc.dram_tensor("maskT", [P, NTB, TQ], bf, kind="ExternalInput")
    Wq = nc.dram_tensor("Wq", [P, KC, GD], bf, kind="ExternalInput")
    Wk = nc.dram_tensor("Wk", [P, KC, GD], bf, kind="ExternalInput")
    Wv = nc.dram_tensor("Wv", [P, KC, GD], bf, kind="ExternalInput")
    Wo = nc.dram_tensor("Wo", [P, GH, D], bf, kind="ExternalInput")
    bq = nc.dram_tensor("bq", [GD], f32, kind="ExternalInput")
    bk = nc.dram_tensor("bk", [GD], f32, kind="ExternalInput")
    bv = nc.dram_tensor("bv", [GD], f32, kind="ExternalInput")
    bo = nc.dram_tensor("bo", [D], f32, kind="ExternalInput")
    out = nc.dram_tensor("out", [P, NOB, TQ], bf, kind="ExternalOutput")

    with TileContext(nc) as tc:
        with ExitStack() as ctx:
            persist = ctx.enter_context(tc.tile_pool(name="persist", bufs=1))
            kvchunk = ctx.enter_context(tc.tile_pool(name="kvchunk", bufs=3))
            work = ctx.enter_context(tc.tile_pool(name="work", bufs=3))
            outp = ctx.enter_context(tc.tile_pool(name="outp", bufs=2))
            # PSUM budget (8 banks): ppool 2x[P,2,TQ] = 4, upool 2x[P,TQ]
            # = 2, dpool 2x[P,TQ] = 2.
            ppool = ctx.enter_context(
                tc.tile_pool(name="ppool", bufs=2, space="PSUM"))
            upool = ctx.enter_context(
                tc.tile_pool(name="upool", bufs=2, space="PSUM"))
            dpool = ctx.enter_context(
                tc.tile_pool(name="dpool", bufs=2, space="PSUM"))

            # ---- constants / weights / biases -------------------------
            # Wq+xq first (whole tensors: 8KB-per-partition descriptors)
            # so the Q projection starts ~6us in, then Wk/kv0/Wv/kv1;
            # mask/Wo are only needed later.
            wq_sb = persist.tile([P, KC, GD], bf)
            xq_sb = persist.tile([P, KC, TQ], bf)
            for q in range(KC):
                nc.sync.dma_start(wq_sb[:, q:q + 1, :],
                                  Wq.ap()[:, q:q + 1, :])
                nc.sync.dma_start(xq_sb[:, q:q + 1, :],
                                  xqT.ap()[:, q:q + 1, :])

            bq_sb = persist.tile([P, GH], f32)
            bk_sb = persist.tile([P, GH], f32)
            nc.sync.dma_start(bq_sb[:], bq.ap().rearrange("(h p) -> p h", p=P))
            nc.sync.dma_start(bk_sb[:], bk.ap().rearrange("(h p) -> p h", p=P))
            bv_row = persist.tile([1, GD], f32)
            nc.sync.dma_start(bv_row[:], bv.ap().unsqueeze(0))
            bv_rep = persist.tile([P, GD], f32)
            nc.gpsimd.partition_broadcast(bv_rep[:], bv_row[:])

            ones_bf = persist.tile([P, 1], bf)
            nc.vector.memset(ones_bf[:], 1.0)

            wk_sb = persist.tile([P, KC, GD], bf)
            wv_sb = persist.tile([P, KC, GD], bf)
            kv_tiles = {}

            def load_kv_chunk(tcknk):
                t = kvchunk.tile([P, KC, 512], bf, name="xkv_t", tag="xkv")
                nc.sync.dma_start(t[:], xkvT.ap()[:, tcknk, :, :])
                kv_tiles[tcknk] = t

            for q in range(4):
                nc.sync.dma_start(wk_sb[:, 2 * q:2 * q + 2, :],
                                  Wk.ap()[:, 2 * q:2 * q + 2, :])
            load_kv_chunk(0)
            for q in range(4):
                nc.sync.dma_start(wv_sb[:, 2 * q:2 * q + 2, :],
                                  Wv.ap()[:, 2 * q:2 * q + 2, :])
            load_kv_chunk(1)

            # ---- Q^T = Wq_g^T x_q^T  (+bq) ----------------------------
            qt_sb = persist.tile([P, GH, TQ], bf)
            for db in range(GH):
                ps = ppool.tile([P, 2, TQ], f32, name="proj_ps",
                                tag="big")[:, 0, :]
                for kc in range(KC):
                    nc.tensor.matmul(ps[:], wq_sb[:, kc, db * P:(db + 1) * P],
                                     xq_sb[:, kc, :],
                                     start=(kc == 0), stop=(kc == KC - 1))
                nc.vector.tensor_tensor(
                    qt_sb[:, db, :], ps[:],
                    bq_sb[:, db:db + 1].to_broadcast([P, TQ]), OP.add)

            # ---- K^T and V over T-chunks ------------------------------
            kt_sb = persist.tile([P, GH, TKV], bf)
            v_sb = persist.tile([P, NTB, GD], bf)
            mask_sb = persist.tile([P, NTB, TQ], bf)
            bo_sb = persist.tile([P, NOB], f32)
            wo_sb = persist.tile([P, GH, D], bf)
            for tcknk in range(NTC):
                if tcknk + 2 < NTC:
                    load_kv_chunk(tcknk + 2)
                xkv_t = kv_tiles.pop(tcknk)
                if tcknk == 1:
                    # queue the bulk "later-phase" loads behind chunks 0-1
                    nc.sync.dma_start(mask_sb[:], maskT.ap())
                    nc.sync.dma_start(wo_sb[:], Wo.ap())
                    nc.sync.dma_start(
                        bo_sb[:], bo.ap().rearrange("(ob p) -> p ob", p=P))
                for db in range(GH):
                    ps = ppool.tile([P, 2, TQ], f32, name="proj_ps",
                                    tag="big")[:, 0, :]
                    for kc in range(KC):
                        nc.tensor.matmul(ps[:], wk_sb[:, kc, db * P:(db + 1) * P],
                                         xkv_t[:, kc, :],
                                         start=(kc == 0), stop=(kc == KC - 1))
                    nc.vector.tensor_tensor(
                        kt_sb[:, db, tcknk * 512:(tcknk + 1) * 512], ps[:],
                        bk_sb[:, db:db + 1].to_broadcast([P, 512]), OP.add)
                for tb in range(4):
                    ps = ppool.tile([P, 2, TQ], f32, name="proj_ps",
                                    tag="big")[:, 0, :]
                    for kc in range(KC):
                        nc.tensor.matmul(ps[:],
                                         xkv_t[:, kc, tb * P:(tb + 1) * P],
                                         wv_sb[:, kc, :],
                                         start=(kc == 0), stop=(kc == KC - 1))
                    nc.vector.tensor_tensor(
                        v_sb[:, tcknk * 4 + tb, :], ps[:], bv_rep[:], OP.add)

            # ---- attention, software-pipelined double-step loop -------
            # Two T-blocks per step: two S-matmuls fill the two banks of
            # one [P, 2, TQ] psum tile, then ONE wide exp + mask-mult.
            # exp/mult run one step AHEAD of the U matmuls that consume
            # them; S prefetch runs two ahead.  Tensor order per step is
            # U (deps long ready), den ones-matmuls, then the next S
            # pair.  With p_t ready a full step early the PE's LDWEIGHTS
            # prefetch is never semaphore-blocked.
            ut_sb = persist.tile([P, GH, TQ], bf)
            NDS = GH * NTB // 2
            s_tiles, p_tiles, p01_tiles = {}, {}, {}
            u_tiles = [None] * GH
            den_tiles = [None] * GH
            fin_pend = None

            def s2_mm(ds):
                t2 = ppool.tile([P, 2, TQ], f32, name="s2_ps", tag="big")
                for k in range(2):
                    h, j = divmod(ds * 2 + k, NTB)
                    nc.tensor.matmul(t2[:, k, :],
                                     kt_sb[:, h, j * P:(j + 1) * P],
                                     qt_sb[:, h, :], start=True, stop=True)
                return t2

            def exp_mult(ds):
                h, j0 = divmod(ds * 2, NTB)
                t2 = s_tiles.pop(ds)
                praw = work.tile([P, 2, TQ], bf, tag="praw", bufs=2)
                nc.scalar.activation(praw[:], t2[:], AF.Exp, scale=SCALE)
                p_t = work.tile([P, 2, TQ], bf, tag="p_t", bufs=4)
                nc.vector.tensor_tensor(p_t[:], praw[:],
                                        mask_sb[:, j0:j0 + 2, :], OP.mult)
                p_tiles[ds] = p_t
                # pair-sum the two P blocks on DVE (bf16 2x rate, and it
                # has ~350ns/step of slack) so den needs ONE ones-matmul
                # per step instead of two on the pacing tensor engine
                p01 = work.tile([P, TQ], bf, tag="p01", bufs=3)
                nc.vector.tensor_tensor(p01[:], p_t[:, 0, :], p_t[:, 1, :],
                                        OP.add)
                p01_tiles[ds] = p01

            def fin_chain(h, drow):
                # 1/den broadcast + normalize.  Returns a list of small
                # thunks so the DVE work can be dripped one op per step
                # (a single fat op at a head boundary blocks the DVE
                # FIFO and stalls the U matmuls behind it).
                rep = work.tile([P, TQ], f32, tag="rep", bufs=2)
                rcp = work.tile([P, TQ], f32, tag="rcp", bufs=2)
                u_ps = u_tiles[h]
                ops = [lambda: nc.gpsimd.partition_broadcast(rep[:], drow[:])]
                for q in range(4):
                    sl = slice(q * TQ // 4, (q + 1) * TQ // 4)
                    ops.append(lambda sl=sl: nc.vector.reciprocal_approx_fast(
                        rcp[:, sl], rep[:, sl]))
                for g in range(2):
                    sl = slice(g * TQ // 2, (g + 1) * TQ // 2)
                    ops.append(lambda sl=sl: nc.vector.tensor_tensor(
                        ut_sb[:, h, sl], u_ps[:, sl], rcp[:, sl], OP.mult))
                return ops

            # prologue
            s_tiles[0] = s2_mm(0)
            s_tiles[1] = s2_mm(1)
            exp_mult(0)
            exp_mult(1)
            fin_ops = []

            for ds in range(NDS):
                h, j0 = divmod(ds * 2, NTB)
                if j0 == 0:
                    u_tiles[h] = upool.tile([P, TQ], f32, name="u_ps",
                                            tag="u_ps")
                    den_tiles[h] = dpool.tile([P, TQ], f32, name="den_ps",
                                              tag="den_ps")
                if fin_ops:
                    fin_ops.pop(0)()  # drip one finalize op per step
                p_t = p_tiles.pop(ds)
                # Grouped same-type pairs (U,U,den,den,S,S) measure
                # ~150ns/step faster than any interleaving of U/den/S.
                for k in range(2):
                    j = j0 + k
                    nc.tensor.matmul(u_tiles[h][:],
                                     v_sb[:, j, h * P:(h + 1) * P],
                                     p_t[:, k, :],
                                     start=(j == 0), stop=(j == NTB - 1))
                nc.tensor.matmul(den_tiles[h][0:1, :], ones_bf[:],
                                 p01_tiles.pop(ds)[:],
                                 start=(j0 == 0), stop=(j0 == NTB - 2))
                if ds + 2 < NDS:
                    s_tiles[ds + 2] = s2_mm(ds + 2)
                    # exp/mask-mult two steps ahead, right behind their
                    # S matmuls: ACT and DVE get a full step of slack
                    exp_mult(ds + 2)
                if j0 == NTB - 2:
                    # den row -> sbuf on the idle ACT engine; +1e-30 is
                    # the all-masked-row guard, folded in for free
                    drow = work.tile([1, TQ], f32, tag="drow", bufs=2)
                    nc.scalar.activation(drow[:], den_tiles[h][0:1, :],
                                         AF.Copy, bias=1e-30)
                    fin_pend = (h, drow)
                if fin_pend is not None and j0 == 0 and ds > 0:
                    fin_ops = fin_chain(*fin_pend)
                    fin_pend = None

            # ---- out^T = Wo_g^T U^T (+bo), direct DMA out -------------
            # Partial (head-group) output; the host sums the pair.
            # Phase 1 accumulates heads 0..2 for 6 o-blocks across every
            # free psum bank while head 3's finalize chain runs on
            # gpsimd/DVE; phase 2 adds head 3 and the last 2 o-blocks.
            o_halves = [outp.tile([P, NOB // 2, TQ], bf, name="o_half",
                                  tag="o_half") for _ in range(2)]
            ppA = ppool.tile([P, 2, TQ], f32, name="o_ps", tag="big")
            ppB = ppool.tile([P, 2, TQ], f32, name="o_ps", tag="big")
            ogrp = [ppA[:, 0, :], ppA[:, 1, :], ppB[:, 0, :], ppB[:, 1, :],
                    upool.tile([P, TQ], f32, name="o_ps", tag="u_ps")[:],
                    dpool.tile([P, TQ], f32, name="o_ps", tag="den_ps")[:]]
            for oi in range(6):
                for hc in range(GH - 1):
                    nc.tensor.matmul(ogrp[oi],
                                     wo_sb[:, hc, oi * P:(oi + 1) * P],
                                     ut_sb[:, hc, :],
                                     start=(hc == 0), stop=False)
            # head 3 finalize; overlaps the phase-1 matmuls above
            for op in fin_chain(*fin_pend):
                op()
            fin_pend = None

            def o_emit(oi, ps):
                # alternate the psum->sbuf bias-add between DVE and ACT
                # (bo is per-partition in the out^T layout) so the tail
                # adds don't serialize on one engine
                if oi % 2 == 0:
                    nc.vector.tensor_tensor(
                        o_halves[oi // 4][:, oi % 4, :], ps,
                        bo_sb[:, oi:oi + 1].to_broadcast([P, TQ]), OP.add)
                else:
                    nc.scalar.add(o_halves[oi // 4][:, oi % 4, :], ps,
                                  bo_sb[:, oi:oi + 1])

            for oi in range(6):
                nc.tensor.matmul(ogrp[oi],
                                 wo_sb[:, GH - 1, oi * P:(oi + 1) * P],
                                 ut_sb[:, GH - 1, :], start=False, stop=True)
                o_emit(oi, ogrp[oi])
            for oi in (6, 7):
                ps = ppool.tile([P, 2, TQ], f32, name="o_ps",
                                tag="big")[:, 0, :]
                for hc in range(GH):
                    nc.tensor.matmul(ps[:],
                                     wo_sb[:, hc, oi * P:(oi + 1) * P],
                                     ut_sb[:, hc, :],
                                     start=(hc == 0), stop=(hc == GH - 1))
                o_emit(oi, ps[:])
            for pair in range(4):
                nc.sync.dma_start(
                    out.ap()[:, 2 * pair:2 * pair + 2, :],
                    o_halves[pair // 2][:, 2 * (pair % 2):2 * (pair % 2) + 2, :])

    nc.finalize()
    return nc


def _shard_inputs(inputs_q, inputs_kv, attention_mask, Wq, bq, Wk, bk, Wv, bv,
                  Wo, bo):
    bf16 = ml_dtypes.bfloat16
    f32 = np.float32

    def ptile(a2d, inner):
        """[R, C] row-major -> [P, R//P, C] partition-major, contiguous."""
        r, c = a2d.shape
        return np.ascontiguousarray(
            a2d.reshape(r // P, P, c).transpose(1, 0, 2)).astype(inner)

    in_maps = []
    xqT = [ptile(inputs_q[b].T, bf16) for b in range(B)]          # [P,KC,TQ]
    xkvT = [ptile(inputs_kv[b].T, bf16)                           # [P,NTC,KC,512]
            .reshape(P, KC, NTC, 512).transpose(0, 2, 1, 3).copy()
            for b in range(B)]
    maskT = [ptile(attention_mask[b].T.astype(np.float32), bf16)  # [P,NTB,TQ]
             for b in range(B)]
    for c in range(NCORES):
        b, g = c % B, c // B  # pair = (b, b+4)
        sl = slice(g * GD, (g + 1) * GD)
        in_maps.append({
            "xqT": xqT[b],
            "xkvT": xkvT[b],
            "maskT": maskT[b],
            "Wq": ptile(np.ascontiguousarray(Wq[:, sl]), bf16),
            "Wk": ptile(np.ascontiguousarray(Wk[:, sl]), bf16),
            "Wv": ptile(np.ascontiguousarray(Wv[:, sl]), bf16),
            "Wo": ptile(np.ascontiguousarray(Wo[sl, :]), bf16),
            "bq": np.ascontiguousarray(bq[sl]).astype(f32),
            "bk": np.ascontiguousarray(bk[sl]).astype(f32),
            "bv": np.ascontiguousarray(bv[sl]).astype(f32),
            "bo": (bo.astype(f32) if g == 0 else np.zeros(D, f32)),
        })
    return in_maps


def kernel(_trace=False, **inputs):
    global _CACHED_NC
    from concourse import bass_utils

    arrs = {k: np.asarray(v) for k, v in inputs.items()}
    in_maps = _shard_inputs(**arrs)

    if _CACHED_NC is None:
        _CACHED_NC = _build_nc()

    res = bass_utils.run_bass_kernel_spmd(
        _CACHED_NC, in_maps, core_ids=list(range(NCORES)), trace=_trace)

    full = np.empty((B, TQ, D), np.float32)
    for b in range(B):
        # pair (b, b+4): sum the two head-group partial outputs on the
        # host (the unshard step for a sum-sharded output)
        outT = (res.results[b]["out"].astype(np.float32)
                + res.results[b + 4]["out"].astype(np.float32))
        full[b] = outT.transpose(1, 0, 2).reshape(D, TQ).T
    if _trace:
        return full, res
    return full


# revision 38
# speedup vs baseline: 1.2735x; 1.0007x over previous
"""Distributed attention kernel for one TRN2 chip (8 NeuronCores).

Problem: multi-head cross-attention
  B=4, TQ=512, TKV=4096, D=1024, H=8 heads (head_dim=128)

Sharding (data-parallel x tensor-parallel, per the hint):
  core c in 0..7 -> (batch b = c % 4, head-group g = c // 4)
  Each core computes heads [4g, 4g+4) for its batch: Wq/Wk/Wv column
  shards, Wo row shard.  Each core writes its full partial out^T; the
  host sums the (c, c+4) pair during the gather (the unshard step for a
  sum-sharded output), so no on-device collective / rendezvous tail.

Device layout (per core; everything transposed so no on-device
transposes are needed - the host passes x^T and mask^T):
  Q^T[dh, t]  = Wq_g^T x_q^T          (4 head-blocks x 8 k-chunks)
  K^T[dh, T]  = Wk_g^T x_kv^T
  V[T, dh]    = x_kv Wv_g             (from x_kv^T chunks as lhsT)
  S^T[T, t]   = K^T_h(block)^T Q^T_h  per head, 32 T-blocks
  P^T         = exp(S^T/sqrt(128)) * mask^T   (no max-subtraction needed:
                scores are O(1) so exp cannot overflow/underflow)
  U^T[dh, t] += V_h(block)^T P^T      accumulated over T-blocks in PSUM
  den        += ones^T P^T            per-block M=1 matmuls into psum
                row 0 (softmax denominators for all t at once)
  U^T *= 1/(den+tiny)                 approx-reciprocal; rows with an
                all-false mask give U = 0 exactly, matching the
                reference's post-softmax wipe
  out^T[o, t] = Wo_g^T U^T (+ bo on group 0 only), DMA out per pair.

Attention loop is software-pipelined: exp+mask-mult for step ds+2
issue right behind that step's S matmuls, so ACT/DVE run a full step
ahead and the PE's semaphore waits are pre-satisfied (LDWEIGHTS
prefetch hides behind streaming).  Per-head finalize (broadcast,
reciprocal quarters, normalize halves) is dripped one op per step into
the next head so it never blocks an engine FIFO; the last head's
finalize overlaps the first 18 output-projection matmuls.

Matmul inputs are bf16 (PE 4x faster than fp32); PSUM accumulation,
softmax denominators and reciprocal stay fp32.
"""

import sys

if "/opt/trn_rl_repo" not in sys.path:
    sys.path.insert(0, "/opt/trn_rl_repo")

import numpy as np
import ml_dtypes
from contextlib import ExitStack

B, TQ, TKV, D, H = 4, 512, 4096, 1024, 8
HD = D // H            # 128 head dim
NCORES = 8
GH = H // 2            # heads per core = 4
GD = GH * HD           # 512 cols per head-group
P = 128
KC = D // P            # 8 contraction chunks
NTB = TKV // P         # 32 T-blocks
NTC = TKV // 512       # 8 T-chunks (DMA granularity)
NOB = D // P           # 8 output o-blocks
SCALE = float(1.0 / np.sqrt(HD))

_CACHED_NC = None


def _build_nc():
    from concourse import mybir, bacc
    from concourse.tile import TileContext

    bf = mybir.dt.bfloat16
    f32 = mybir.dt.float32
    AF = mybir.ActivationFunctionType
    OP = mybir.AluOpType

    nc = bacc.Bacc("TRN2", target_bir_lowering=False, debug=False,
                   num_devices=NCORES)

    # All inputs are pre-tiled on the host into partition-major layouts
    # so every DMA is 128 contiguous multi-KB descriptors.
    xqT = nc.dram_tensor("xqT", [P, KC, TQ], bf, kind="ExternalInput")
    xkvT = nc.dram_tensor("xkvT", [P, NTC, KC, 512], bf, kind="ExternalInput")
    maskT = nc.dram_tensor("maskT", [P, NTB, TQ], bf, kind="ExternalInput")
    Wq = nc.dram_tensor("Wq", [P, KC, GD], bf, kind="ExternalInput")
    Wk = nc.dram_tensor("Wk", [P, KC, GD], bf, kind="ExternalInput")
    Wv = nc.dram_tensor("Wv", [P, KC, GD], bf, kind="ExternalInput")
    Wo = nc.dram_tensor("Wo", [P, GH, D], bf, kind="ExternalInput")
    bq = nc.dram_tensor("bq", [GD], f32, kind="ExternalInput")
    bk = nc.dram_tensor("bk", [GD], f32, kind="ExternalInput")
    bv = nc.dram_tensor("bv", [GD], f32, kind="ExternalInput")
    bo = nc.dram_tensor("bo", [D], f32, kind="ExternalInput")
    out = nc.dram_tensor("out", [P, NOB, TQ], bf, kind="ExternalOutput")

    with TileContext(nc) as tc:
        with ExitStack() as ctx:
            persist = ctx.enter_context(tc.tile_pool(name="persist", bufs=1))
            kvchunk = ctx.enter_context(tc.tile_pool(name="kvchunk", bufs=3))
            work = ctx.enter_context(tc.tile_pool(name="work", bufs=3))
            outp = ctx.enter_context(tc.tile_pool(name="outp", bufs=2))
            # PSUM budget (8 banks): ppool 2x[P,2,TQ] = 4, upool 2x[P,TQ]
            # = 2, dpool 2x[P,TQ] = 2.
            ppool = ctx.enter_context(
                tc.tile_pool(name="ppool", bufs=2, space="PSUM"))
            upool = ctx.enter_context(
                tc.tile_pool(name="upool", bufs=2, space="PSUM"))
            dpool = ctx.enter_context(
                tc.tile_pool(name="dpool", bufs=2, space="PSUM"))

            # ---- constants / weights / biases -------------------------
            # Wq+xq first (whole tensors: 8KB-per-partition descriptors)
            # so the Q projection starts ~6us in, then Wk/kv0/Wv/kv1;
            # mask/Wo are only needed later.
            wq_sb = persist.tile([P, KC, GD], bf)
            xq_sb = persist.tile([P, KC, TQ], bf)
            for q in range(KC):
                nc.sync.dma_start(wq_sb[:, q:q + 1, :],
                                  Wq.ap()[:, q:q + 1, :])
                nc.sync.dma_start(xq_sb[:, q:q + 1, :],
                                  xqT.ap()[:, q:q + 1, :])

            bq_sb = persist.tile([P, GH], f32)
            bk_sb = persist.tile([P, GH], f32)
            nc.sync.dma_start(bq_sb[:], bq.ap().rearrange("(h p) -> p h", p=P))
            nc.sync.dma_start(bk_sb[:], bk.ap().rearrange("(h p) -> p h", p=P))
            bv_row = persist.tile([1, GD], f32)
            nc.sync.dma_start(bv_row[:], bv.ap().unsqueeze(0))
            bv_rep = persist.tile([P, GD], f32)
            nc.gpsimd.partition_broadcast(bv_rep[:], bv_row[:])

            ones_bf = persist.tile([P, 1], bf)
            nc.vector.memset(ones_bf[:], 1.0)

            wk_sb = persist.tile([P, KC, GD], bf)
            wv_sb = persist.tile([P, KC, GD], bf)
            kv_tiles = {}

            def load_kv_chunk(tcknk, split=1):
                # chunks 0-1 are needed ~25us in but a 1MB dma_start
                # lands on a single queue (~6us + queue-init); split the
                # early ones across queues
                t = kvchunk.tile([P, KC, 512], bf, name="xkv_t", tag="xkv")
                n = KC // split
                for piece in range(split):
                    nc.sync.dma_start(
                        t[:, piece * n:(piece + 1) * n, :],
                        xkvT.ap()[:, tcknk, piece * n:(piece + 1) * n, :])
                kv_tiles[tcknk] = t

            for q in range(4):
                nc.sync.dma_start(wk_sb[:, 2 * q:2 * q + 2, :],
                                  Wk.ap()[:, 2 * q:2 * q + 2, :])
            load_kv_chunk(0, split=4)
            for q in range(4):
                nc.sync.dma_start(wv_sb[:, 2 * q:2 * q + 2, :],
                                  Wv.ap()[:, 2 * q:2 * q + 2, :])
            load_kv_chunk(1, split=4)

            # ---- Q^T = Wq_g^T x_q^T  (+bq) ----------------------------
            qt_sb = persist.tile([P, GH, TQ], bf)
            for db in range(GH):
                ps = ppool.tile([P, 2, TQ], f32, name="proj_ps",
                                tag="big")[:, 0, :]
                for kc in range(KC):
                    nc.tensor.matmul(ps[:], wq_sb[:, kc, db * P:(db + 1) * P],
                                     xq_sb[:, kc, :],
                                     start=(kc == 0), stop=(kc == KC - 1))
                nc.vector.tensor_tensor(
                    qt_sb[:, db, :], ps[:],
                    bq_sb[:, db:db + 1].to_broadcast([P, TQ]), OP.add)

            # ---- K^T and V over T-chunks ------------------------------
            kt_sb = persist.tile([P, GH, TKV], bf)
            v_sb = persist.tile([P, NTB, GD], bf)
            mask_sb = persist.tile([P, NTB, TQ], bf)
            bo_sb = persist.tile([P, NOB], f32)
            wo_sb = persist.tile([P, GH, D], bf)
            for tcknk in range(NTC):
                if tcknk + 2 < NTC:
                    load_kv_chunk(tcknk + 2)
                xkv_t = kv_tiles.pop(tcknk)
                if tcknk == 1:
                    # queue the bulk "later-phase" loads behind chunks 0-1
                    nc.sync.dma_start(mask_sb[:], maskT.ap())
                    nc.sync.dma_start(wo_sb[:], Wo.ap())
                    nc.sync.dma_start(
                        bo_sb[:], bo.ap().rearrange("(ob p) -> p ob", p=P))
                for db in range(GH):
                    ps = ppool.tile([P, 2, TQ], f32, name="proj_ps",
                                    tag="big")[:, 0, :]
                    for kc in range(KC):
                        nc.tensor.matmul(ps[:], wk_sb[:, kc, db * P:(db + 1) * P],
                                         xkv_t[:, kc, :],
                                         start=(kc == 0), stop=(kc == KC - 1))
                    nc.vector.tensor_tensor(
                        kt_sb[:, db, tcknk * 512:(tcknk + 1) * 512], ps[:],
                        bk_sb[:, db:db + 1].to_broadcast([P, 512]), OP.add)
                for tb in range(4):
                    ps = ppool.tile([P, 2, TQ], f32, name="proj_ps",
                                    tag="big")[:, 0, :]
                    for kc in range(KC):
                        nc.tensor.matmul(ps[:],
                                         xkv_t[:, kc, tb * P:(tb + 1) * P],
                                         wv_sb[:, kc, :],
                                         start=(kc == 0), stop=(kc == KC - 1))
                    nc.vector.tensor_tensor(
                        v_sb[:, tcknk * 4 + tb, :], ps[:], bv_rep[:], OP.add)

            # ---- attention, software-pipelined double-step loop -------
            # Two T-blocks per step: two S-matmuls fill the two banks of
            # one [P, 2, TQ] psum tile, then ONE wide exp + mask-mult.
            # exp/mult run one step AHEAD of the U matmuls that consume
            # them; S prefetch runs two ahead.  Tensor order per step is
            # U (deps long ready), den ones-matmuls, then the next S
            # pair.  With p_t ready a full step early the PE's LDWEIGHTS
            # prefetch is never semaphore-blocked.
            ut_sb = persist.tile([P, GH, TQ], bf)
            NDS = GH * NTB // 2
            s_tiles, p_tiles, p01_tiles = {}, {}, {}
            u_tiles = [None] * GH
            den_tiles = [None] * GH
            fin_pend = None

            def s2_mm(ds):
                t2 = ppool.tile([P, 2, TQ], f32, name="s2_ps", tag="big")
                for k in range(2):
                    h, j = divmod(ds * 2 + k, NTB)
                    nc.tensor.matmul(t2[:, k, :],
                                     kt_sb[:, h, j * P:(j + 1) * P],
                                     qt_sb[:, h, :], start=True, stop=True)
                return t2

            def exp_mult(ds):
                h, j0 = divmod(ds * 2, NTB)
                t2 = s_tiles.pop(ds)
                praw = work.tile([P, 2, TQ], bf, tag="praw", bufs=2)
                nc.scalar.activation(praw[:], t2[:], AF.Exp, scale=SCALE)
                p_t = work.tile([P, 2, TQ], bf, tag="p_t", bufs=4)
                nc.vector.tensor_tensor(p_t[:], praw[:],
                                        mask_sb[:, j0:j0 + 2, :], OP.mult)
                p_tiles[ds] = p_t
                # pair-sum the two P blocks on DVE (bf16 2x rate, and it
                # has ~350ns/step of slack) so den needs ONE ones-matmul
                # per step instead of two on the pacing tensor engine
                p01 = work.tile([P, TQ], bf, tag="p01", bufs=3)
                nc.vector.tensor_tensor(p01[:], p_t[:, 0, :], p_t[:, 1, :],
                                        OP.add)
                p01_tiles[ds] = p01

            def fin_chain(h, drow):
                # 1/den broadcast + normalize.  Returns a list of small
                # thunks so the DVE work can be dripped one op per step
                # (a single fat op at a head boundary blocks the DVE
                # FIFO and stalls the U matmuls behind it).
                rep = work.tile([P, TQ], f32, tag="rep", bufs=2)
                rcp = work.tile([P, TQ], f32, tag="rcp", bufs=2)
                u_ps = u_tiles[h]
                ops = [lambda: nc.gpsimd.partition_broadcast(rep[:], drow[:])]
                for q in range(4):
                    sl = slice(q * TQ // 4, (q + 1) * TQ // 4)
                    ops.append(lambda sl=sl: nc.vector.reciprocal_approx_fast(
                        rcp[:, sl], rep[:, sl]))
                for g in range(2):
                    sl = slice(g * TQ // 2, (g + 1) * TQ // 2)
                    ops.append(lambda sl=sl: nc.vector.tensor_tensor(
                        ut_sb[:, h, sl], u_ps[:, sl], rcp[:, sl], OP.mult))
                return ops

            # prologue
            s_tiles[0] = s2_mm(0)
            s_tiles[1] = s2_mm(1)
            exp_mult(0)
            exp_mult(1)
            fin_ops = []

            for ds in range(NDS):
                h, j0 = divmod(ds * 2, NTB)
                if j0 == 0:
                    u_tiles[h] = upool.tile([P, TQ], f32, name="u_ps",
                                            tag="u_ps")
                    den_tiles[h] = dpool.tile([P, TQ], f32, name="den_ps",
                                              tag="den_ps")
                if fin_ops:
                    fin_ops.pop(0)()  # drip one finalize op per step
                p_t = p_tiles.pop(ds)
                # Grouped same-type pairs (U,U,den,den,S,S) measure
                # ~150ns/step faster than any interleaving of U/den/S.
                for k in range(2):
                    j = j0 + k
                    nc.tensor.matmul(u_tiles[h][:],
                                     v_sb[:, j, h * P:(h + 1) * P],
                                     p_t[:, k, :],
                                     start=(j == 0), stop=(j == NTB - 1))
                nc.tensor.matmul(den_tiles[h][0:1, :], ones_bf[:],
                                 p01_tiles.pop(ds)[:],
                                 start=(j0 == 0), stop=(j0 == NTB - 2))
                if ds + 2 < NDS:
                    s_tiles[ds + 2] = s2_mm(ds + 2)
                    # exp/mask-mult two steps ahead, right behind their
                    # S matmuls: ACT and DVE get a full step of slack
                    exp_mult(ds + 2)
                if j0 == NTB - 2:
                    # den row -> sbuf on the idle ACT engine; +1e-30 is
                    # the all-masked-row guard, folded in for free
                    drow = work.tile([1, TQ], f32, tag="drow", bufs=2)
                    nc.scalar.activation(drow[:], den_tiles[h][0:1, :],
                                         AF.Copy, bias=1e-30)
                    fin_pend = (h, drow)
                if fin_pend is not None and j0 == 0 and ds > 0:
                    fin_ops = fin_chain(*fin_pend)
                    fin_pend = None

            # ---- out^T = Wo_g^T U^T (+bo), direct DMA out -------------
            # Partial (head-group) output; the host sums the pair.
            # Phase 1 accumulates heads 0..2 for 6 o-blocks across every
            # free psum bank while head 3's finalize chain runs on
            # gpsimd/DVE; phase 2 adds head 3 and the last 2 o-blocks.
            o_halves = [outp.tile([P, NOB // 2, TQ], bf, name="o_half",
                                  tag="o_half") for _ in range(2)]
            ppA = ppool.tile([P, 2, TQ], f32, name="o_ps", tag="big")
            ppB = ppool.tile([P, 2, TQ], f32, name="o_ps", tag="big")
            ogrp = [ppA[:, 0, :], ppA[:, 1, :], ppB[:, 0, :], ppB[:, 1, :],
                    upool.tile([P, TQ], f32, name="o_ps", tag="u_ps")[:],
                    dpool.tile([P, TQ], f32, name="o_ps", tag="den_ps")[:]]
            for oi in range(6):
                for hc in range(GH - 1):
                    nc.tensor.matmul(ogrp[oi],
                                     wo_sb[:, hc, oi * P:(oi + 1) * P],
                                     ut_sb[:, hc, :],
                                     start=(hc == 0), stop=False)
            # head 3 finalize; overlaps the phase-1 matmuls above
            for op in fin_chain(*fin_pend):
                op()
            fin_pend = None

            def o_emit(oi, ps):
                # alternate the psum->sbuf bias-add between DVE and ACT
                # (bo is per-partition in the out^T layout) so the tail
                # adds don't serialize on one engine
                if oi % 2 == 0:
                    nc.vector.tensor_tensor(
                        o_halves[oi // 4][:, oi % 4, :], ps,
                        bo_sb[:, oi:oi + 1].to_broadcast([P, TQ]), OP.add)
                else:
                    nc.scalar.add(o_halves[oi // 4][:, oi % 4, :], ps,
                                  bo_sb[:, oi:oi + 1])

            for oi in range(6):
                nc.tensor.matmul(ogrp[oi],
                                 wo_sb[:, GH - 1, oi * P:(oi + 1) * P],
                                 ut_sb[:, GH - 1, :], start=False, stop=True)
                o_emit(oi, ogrp[oi])
            for oi in (6, 7):
                ps = ppool.tile([P, 2, TQ], f32, name="o_ps",
                                tag="big")[:, 0, :]
                for hc in range(GH):
                    nc.tensor.matmul(ps[:],
                                     wo_sb[:, hc, oi * P:(oi + 1) * P],
                                     ut_sb[:, hc, :],
                                     start=(hc == 0), stop=(hc == GH - 1))
                o_emit(oi, ps[:])
            for pair in range(4):
                nc.sync.dma_start(
                    out.ap()[:, 2 * pair:2 * pair + 2, :],
                    o_halves[pair // 2][:, 2 * (pair % 2):2 * (pair % 2) + 2, :])

    nc.finalize()
    return nc


def _shard_inputs(inputs_q, inputs_kv, attention_mask, Wq, bq, Wk, bk, Wv, bv,
                  Wo, bo):
    bf16 = ml_dtypes.bfloat16
    f32 = np.float32

    def ptile(a2d, inner):
        """[R, C] row-major -> [P, R//P, C] partition-major, contiguous."""
        r, c = a2d.shape
        return np.ascontiguousarray(
            a2d.reshape(r // P, P, c).transpose(1, 0, 2)).astype(inner)

    in_maps = []
    xqT = [ptile(inputs_q[b].T, bf16) for b in range(B)]          # [P,KC,TQ]
    xkvT = [ptile(inputs_kv[b].T, bf16)                           # [P,NTC,KC,512]
            .reshape(P, KC, NTC, 512).transpose(0, 2, 1, 3).copy()
            for b in range(B)]
    maskT = [ptile(attention_mask[b].T.astype(np.float32), bf16)  # [P,NTB,TQ]
             for b in range(B)]
    for c in range(NCORES):
        b, g = c % B, c // B  # pair = (b, b+4)
        sl = slice(g * GD, (g + 1) * GD)
        in_maps.append({
            "xqT": xqT[b],
            "xkvT": xkvT[b],
            "maskT": maskT[b],
            "Wq": ptile(np.ascontiguousarray(Wq[:, sl]), bf16),
            "Wk": ptile(np.ascontiguousarray(Wk[:, sl]), bf16),
            "Wv": ptile(np.ascontiguousarray(Wv[:, sl]), bf16),
            "Wo": ptile(np.ascontiguousarray(Wo[sl, :]), bf16),
            "bq": np.ascontiguousarray(bq[sl]).astype(f32),
            "bk": np.ascontiguousarray(bk[sl]).astype(f32),
            "bv": np.ascontiguousarray(bv[sl]).astype(f32),
            "bo": (bo.astype(f32) if g == 0 else np.zeros(D, f32)),
        })
    return in_maps


def kernel(_trace=False, **inputs):
    global _CACHED_NC
    from concourse import bass_utils

    arrs = {k: np.asarray(v) for k, v in inputs.items()}
    in_maps = _shard_inputs(**arrs)

    if _CACHED_NC is None:
        _CACHED_NC = _build_nc()

    res = bass_utils.run_bass_kernel_spmd(
        _CACHED_NC, in_maps, core_ids=list(range(NCORES)), trace=_trace)

    full = np.empty((B, TQ, D), np.float32)
    for b in range(B):
        # pair (b, b+4): sum the two head-group partial outputs on the
        # host (the unshard step for a sum-sharded output)
        outT = (res.results[b]["out"].astype(np.float32)
                + res.results[b + 4]["out"].astype(np.float32))
        full[b] = outT.transpose(1, 0, 2).reshape(D, TQ).T
    if _trace:
        return full, res
    return full


# revision 39
# speedup vs baseline: 1.2868x; 1.0104x over previous
"""Distributed attention kernel for one TRN2 chip (8 NeuronCores).

Problem: multi-head cross-attention
  B=4, TQ=512, TKV=4096, D=1024, H=8 heads (head_dim=128)

Sharding (data-parallel x tensor-parallel, per the hint):
  core c in 0..7 -> (batch b = c % 4, head-group g = c // 4)
  Each core computes heads [4g, 4g+4) for its batch: Wq/Wk/Wv column
  shards, Wo row shard.  Each core writes its full partial out^T; the
  host sums the (c, c+4) pair during the gather (the unshard step for a
  sum-sharded output), so no on-device collective / rendezvous tail.

Device layout (per core; everything transposed so no on-device
transposes are needed - the host passes x^T and mask^T):
  Q^T[dh, t]  = Wq_g^T x_q^T          (4 head-blocks x 8 k-chunks)
  K^T[dh, T]  = Wk_g^T x_kv^T
  V[T, dh]    = x_kv Wv_g             (from x_kv^T chunks as lhsT)
  S^T[T, t]   = K^T_h(block)^T Q^T_h  per head, 32 T-blocks
  P^T         = exp(S^T/sqrt(128)) * mask^T   (no max-subtraction needed:
                scores are O(1) so exp cannot overflow/underflow)
  U^T[dh, t] += V_h(block)^T P^T      accumulated over T-blocks in PSUM
  den        += ones^T P^T            per-block M=1 matmuls into psum
                row 0 (softmax denominators for all t at once)
  U^T *= 1/(den+tiny)                 approx-reciprocal; rows with an
                all-false mask give U = 0 exactly, matching the
                reference's post-softmax wipe
  out^T[o, t] = Wo_g^T U^T (+ bo on group 0 only), DMA out per pair.

Attention loop is software-pipelined: exp+mask-mult for step ds+2
issue right behind that step's S matmuls, so ACT/DVE run a full step
ahead and the PE's semaphore waits are pre-satisfied (LDWEIGHTS
prefetch hides behind streaming).  Per-head finalize (broadcast,
reciprocal quarters, normalize halves) is dripped one op per step into
the next head so it never blocks an engine FIFO; the last head's
finalize overlaps the first 18 output-projection matmuls.

Matmul inputs are bf16 (PE 4x faster than fp32); PSUM accumulation,
softmax denominators and reciprocal stay fp32.
"""

import sys

if "/opt/trn_rl_repo" not in sys.path:
    sys.path.insert(0, "/opt/trn_rl_repo")

import numpy as np
import ml_dtypes
from contextlib import ExitStack

B, TQ, TKV, D, H = 4, 512, 4096, 1024, 8
HD = D // H            # 128 head dim
NCORES = 8
GH = H // 2            # heads per core = 4
GD = GH * HD           # 512 cols per head-group
P = 128
KC = D // P            # 8 contraction chunks
NTB = TKV // P         # 32 T-blocks
NTC = TKV // 512       # 8 T-chunks (DMA granularity)
NOB = D // P           # 8 output o-blocks
SCALE = float(1.0 / np.sqrt(HD))

_CACHED_NC = None


def _build_nc():
    from concourse import mybir, bacc
    from concourse.tile import TileContext

    bf = mybir.dt.bfloat16
    f32 = mybir.dt.float32
    AF = mybir.ActivationFunctionType
    OP = mybir.AluOpType

    nc = bacc.Bacc("TRN2", target_bir_lowering=False, debug=False,
                   num_devices=NCORES)

    # All inputs are pre-tiled on the host into partition-major layouts
    # so every DMA is 128 contiguous multi-KB descriptors.
    xqT = nc.dram_tensor("xqT", [P, KC, TQ], bf, kind="ExternalInput")
    xkvT = nc.dram_tensor("xkvT", [P, NTC, KC, 512], bf, kind="ExternalInput")
    maskT = nc.dram_tensor("maskT", [P, NTB, TQ], bf, kind="ExternalInput")
    Wq = nc.dram_tensor("Wq", [P, KC, GD], bf, kind="ExternalInput")
    Wk = nc.dram_tensor("Wk", [P, KC, GD], bf, kind="ExternalInput")
    Wv = nc.dram_tensor("Wv", [P, KC, GD], bf, kind="ExternalInput")
    Wo = nc.dram_tensor("Wo", [P, GH, D], bf, kind="ExternalInput")
    bq = nc.dram_tensor("bq", [GD], f32, kind="ExternalInput")
    bk = nc.dram_tensor("bk", [GD], f32, kind="ExternalInput")
    bv = nc.dram_tensor("bv", [GD], f32, kind="ExternalInput")
    bo = nc.dram_tensor("bo", [D], f32, kind="ExternalInput")
    out = nc.dram_tensor("out", [P, NOB, TQ], bf, kind="ExternalOutput")

    with TileContext(nc) as tc:
        with ExitStack() as ctx:
            persist = ctx.enter_context(tc.tile_pool(name="persist", bufs=1))
            kvchunk = ctx.enter_context(tc.tile_pool(name="kvchunk", bufs=3))
            work = ctx.enter_context(tc.tile_pool(name="work", bufs=3))
            outp = ctx.enter_context(tc.tile_pool(name="outp", bufs=2))
            # PSUM budget (8 banks): ppool 2x[P,2,TQ] = 4, upool 2x[P,TQ]
            # = 2, dpool 2x[P,TQ] = 2.
            ppool = ctx.enter_context(
                tc.tile_pool(name="ppool", bufs=2, space="PSUM"))
            upool = ctx.enter_context(
                tc.tile_pool(name="upool", bufs=2, space="PSUM"))
            dpool = ctx.enter_context(
                tc.tile_pool(name="dpool", bufs=2, space="PSUM"))

            # ---- constants / weights / biases -------------------------
            # Wq+xq first (whole tensors: 8KB-per-partition descriptors)
            # so the Q projection starts ~6us in, then Wk/kv0/Wv/kv1;
            # mask/Wo are only needed later.
            wq_sb = persist.tile([P, KC, GD], bf)
            xq_sb = persist.tile([P, KC, TQ], bf)
            for q in range(KC):
                nc.sync.dma_start(wq_sb[:, q:q + 1, :],
                                  Wq.ap()[:, q:q + 1, :])
                nc.sync.dma_start(xq_sb[:, q:q + 1, :],
                                  xqT.ap()[:, q:q + 1, :])

            bq_sb = persist.tile([P, GH], f32)
            bk_sb = persist.tile([P, GH], f32)
            nc.sync.dma_start(bq_sb[:], bq.ap().rearrange("(h p) -> p h", p=P))
            nc.sync.dma_start(bk_sb[:], bk.ap().rearrange("(h p) -> p h", p=P))
            bv_row = persist.tile([1, GD], f32)
            nc.sync.dma_start(bv_row[:], bv.ap().unsqueeze(0))
            bv_rep = persist.tile([P, GD], f32)
            nc.gpsimd.partition_broadcast(bv_rep[:], bv_row[:])

            ones_bf = persist.tile([P, 1], bf)
            nc.vector.memset(ones_bf[:], 1.0)

            wk_sb = persist.tile([P, KC, GD], bf)
            wv_sb = persist.tile([P, KC, GD], bf)
            kv_tiles = {}

            def load_kv_chunk(tcknk, split=1):
                # chunks 0-1 are needed ~25us in but a 1MB dma_start
                # lands on a single queue (~6us + queue-init); split the
                # early ones across queues
                t = kvchunk.tile([P, KC, 512], bf, name="xkv_t", tag="xkv")
                n = KC // split
                for piece in range(split):
                    nc.sync.dma_start(
                        t[:, piece * n:(piece + 1) * n, :],
                        xkvT.ap()[:, tcknk, piece * n:(piece + 1) * n, :])
                kv_tiles[tcknk] = t

            for q in range(4):
                nc.sync.dma_start(wk_sb[:, 2 * q:2 * q + 2, :],
                                  Wk.ap()[:, 2 * q:2 * q + 2, :])
            load_kv_chunk(0, split=4)
            for q in range(4):
                nc.sync.dma_start(wv_sb[:, 2 * q:2 * q + 2, :],
                                  Wv.ap()[:, 2 * q:2 * q + 2, :])
            load_kv_chunk(1, split=4)

            # ---- Q^T = Wq_g^T x_q^T  (+bq) ----------------------------
            qt_sb = persist.tile([P, GH, TQ], bf)
            for db in range(GH):
                ps = ppool.tile([P, 2, TQ], f32, name="proj_ps",
                                tag="big")[:, 0, :]
                for kc in range(KC):
                    nc.tensor.matmul(ps[:], wq_sb[:, kc, db * P:(db + 1) * P],
                                     xq_sb[:, kc, :],
                                     start=(kc == 0), stop=(kc == KC - 1))
                nc.vector.tensor_tensor(
                    qt_sb[:, db, :], ps[:],
                    bq_sb[:, db:db + 1].to_broadcast([P, TQ]), OP.add)

            # ---- K^T and V over T-chunks ------------------------------
            kt_sb = persist.tile([P, GH, TKV], bf)
            v_sb = persist.tile([P, NTB, GD], bf)
            mask_sb = persist.tile([P, NTB, TQ], bf)
            bo_sb = persist.tile([P, NOB], f32)
            wo_sb = persist.tile([P, GH, D], bf)
            for tcknk in range(NTC):
                if tcknk + 2 < NTC:
                    load_kv_chunk(tcknk + 2)
                xkv_t = kv_tiles.pop(tcknk)
                if tcknk == 3:
                    # the 4MB mask saturates HBM if issued early; issue
                    # it here - past the urgent wk/wv/chunk loads, still
                    # ~70us before attention needs it
                    nc.sync.dma_start(mask_sb[:], maskT.ap())
                    nc.sync.dma_start(wo_sb[:], Wo.ap())
                    nc.sync.dma_start(
                        bo_sb[:], bo.ap().rearrange("(ob p) -> p ob", p=P))
                for db in range(GH):
                    ps = ppool.tile([P, 2, TQ], f32, name="proj_ps",
                                    tag="big")[:, 0, :]
                    for kc in range(KC):
                        nc.tensor.matmul(ps[:], wk_sb[:, kc, db * P:(db + 1) * P],
                                         xkv_t[:, kc, :],
                                         start=(kc == 0), stop=(kc == KC - 1))
                    nc.vector.tensor_tensor(
                        kt_sb[:, db, tcknk * 512:(tcknk + 1) * 512], ps[:],
                        bk_sb[:, db:db + 1].to_broadcast([P, 512]), OP.add)
                for tb in range(4):
                    ps = ppool.tile([P, 2, TQ], f32, name="proj_ps",
                                    tag="big")[:, 0, :]
                    for kc in range(KC):
                        nc.tensor.matmul(ps[:],
                                         xkv_t[:, kc, tb * P:(tb + 1) * P],
                                         wv_sb[:, kc, :],
                                         start=(kc == 0), stop=(kc == KC - 1))
                    nc.vector.tensor_tensor(
                        v_sb[:, tcknk * 4 + tb, :], ps[:], bv_rep[:], OP.add)

            # ---- attention, software-pipelined double-step loop -------
            # Two T-blocks per step: two S-matmuls fill the two banks of
            # one [P, 2, TQ] psum tile, then ONE wide exp + mask-mult.
            # exp/mult run one step AHEAD of the U matmuls that consume
            # them; S prefetch runs two ahead.  Tensor order per step is
            # U (deps long ready), den ones-matmuls, then the next S
            # pair.  With p_t ready a full step early the PE's LDWEIGHTS
            # prefetch is never semaphore-blocked.
            ut_sb = persist.tile([P, GH, TQ], bf)
            NDS = GH * NTB // 2
            s_tiles, p_tiles, p01_tiles = {}, {}, {}
            u_tiles = [None] * GH
            den_tiles = [None] * GH
            fin_pend = None

            def s2_mm(ds):
                t2 = ppool.tile([P, 2, TQ], f32, name="s2_ps", tag="big")
                for k in range(2):
                    h, j = divmod(ds * 2 + k, NTB)
                    nc.tensor.matmul(t2[:, k, :],
                                     kt_sb[:, h, j * P:(j + 1) * P],
                                     qt_sb[:, h, :], start=True, stop=True)
                return t2

            def exp_mult(ds):
                h, j0 = divmod(ds * 2, NTB)
                t2 = s_tiles.pop(ds)
                praw = work.tile([P, 2, TQ], bf, tag="praw", bufs=2)
                nc.scalar.activation(praw[:], t2[:], AF.Exp, scale=SCALE)
                p_t = work.tile([P, 2, TQ], bf, tag="p_t", bufs=4)
                nc.vector.tensor_tensor(p_t[:], praw[:],
                                        mask_sb[:, j0:j0 + 2, :], OP.mult)
                p_tiles[ds] = p_t
                # pair-sum the two P blocks on DVE (bf16 2x rate, and it
                # has ~350ns/step of slack) so den needs ONE ones-matmul
                # per step instead of two on the pacing tensor engine
                p01 = work.tile([P, TQ], bf, tag="p01", bufs=3)
                nc.vector.tensor_tensor(p01[:], p_t[:, 0, :], p_t[:, 1, :],
                                        OP.add)
                p01_tiles[ds] = p01

            def fin_chain(h, drow):
                # 1/den broadcast + normalize.  Returns a list of small
                # thunks so the DVE work can be dripped one op per step
                # (a single fat op at a head boundary blocks the DVE
                # FIFO and stalls the U matmuls behind it).
                rep = work.tile([P, TQ], f32, tag="rep", bufs=2)
                rcp = work.tile([P, TQ], f32, tag="rcp", bufs=2)
                u_ps = u_tiles[h]
                ops = [lambda: nc.gpsimd.partition_broadcast(rep[:], drow[:])]
                for q in range(4):
                    sl = slice(q * TQ // 4, (q + 1) * TQ // 4)
                    ops.append(lambda sl=sl: nc.vector.reciprocal_approx_fast(
                        rcp[:, sl], rep[:, sl]))
                for g in range(2):
                    sl = slice(g * TQ // 2, (g + 1) * TQ // 2)
                    ops.append(lambda sl=sl: nc.vector.tensor_tensor(
                        ut_sb[:, h, sl], u_ps[:, sl], rcp[:, sl], OP.mult))
                return ops

            # prologue
            s_tiles[0] = s2_mm(0)
            s_tiles[1] = s2_mm(1)
            exp_mult(0)
            exp_mult(1)
            fin_ops = []

            for ds in range(NDS):
                h, j0 = divmod(ds * 2, NTB)
                if j0 == 0:
                    u_tiles[h] = upool.tile([P, TQ], f32, name="u_ps",
                                            tag="u_ps")
                    den_tiles[h] = dpool.tile([P, TQ], f32, name="den_ps",
                                              tag="den_ps")
                if fin_ops:
                    fin_ops.pop(0)()  # drip one finalize op per step
                p_t = p_tiles.pop(ds)
                # Grouped same-type pairs (U,U,den,den,S,S) measure
                # ~150ns/step faster than any interleaving of U/den/S.
                for k in range(2):
                    j = j0 + k
                    nc.tensor.matmul(u_tiles[h][:],
                                     v_sb[:, j, h * P:(h + 1) * P],
                                     p_t[:, k, :],
                                     start=(j == 0), stop=(j == NTB - 1))
                nc.tensor.matmul(den_tiles[h][0:1, :], ones_bf[:],
                                 p01_tiles.pop(ds)[:],
                                 start=(j0 == 0), stop=(j0 == NTB - 2))
                if ds + 2 < NDS:
                    s_tiles[ds + 2] = s2_mm(ds + 2)
                    # exp/mask-mult two steps ahead, right behind their
                    # S matmuls: ACT and DVE get a full step of slack
                    exp_mult(ds + 2)
                if j0 == NTB - 2:
                    # den row -> sbuf on the idle ACT engine; +1e-30 is
                    # the all-masked-row guard, folded in for free
                    drow = work.tile([1, TQ], f32, tag="drow", bufs=2)
                    nc.scalar.activation(drow[:], den_tiles[h][0:1, :],
                                         AF.Copy, bias=1e-30)
                    fin_pend = (h, drow)
                if fin_pend is not None and j0 == 0 and ds > 0:
                    fin_ops = fin_chain(*fin_pend)
                    fin_pend = None

            # ---- out^T = Wo_g^T U^T (+bo), direct DMA out -------------
            # Partial (head-group) output; the host sums the pair.
            # Phase 1 accumulates heads 0..2 for 6 o-blocks across every
            # free psum bank while head 3's finalize chain runs on
            # gpsimd/DVE; phase 2 adds head 3 and the last 2 o-blocks.
            o_halves = [outp.tile([P, NOB // 2, TQ], bf, name="o_half",
                                  tag="o_half") for _ in range(2)]
            ppA = ppool.tile([P, 2, TQ], f32, name="o_ps", tag="big")
            ppB = ppool.tile([P, 2, TQ], f32, name="o_ps", tag="big")
            ogrp = [ppA[:, 0, :], ppA[:, 1, :], ppB[:, 0, :], ppB[:, 1, :],
                    upool.tile([P, TQ], f32, name="o_ps", tag="u_ps")[:],
                    dpool.tile([P, TQ], f32, name="o_ps", tag="den_ps")[:]]
            for oi in range(6):
                for hc in range(GH - 1):
                    nc.tensor.matmul(ogrp[oi],
                                     wo_sb[:, hc, oi * P:(oi + 1) * P],
                                     ut_sb[:, hc, :],
                                     start=(hc == 0), stop=False)
            # head 3 finalize; overlaps the phase-1 matmuls above
            for op in fin_chain(*fin_pend):
                op()
            fin_pend = None

            def o_emit(oi, ps):
                # alternate the psum->sbuf bias-add between DVE and ACT
                # (bo is per-partition in the out^T layout) so the tail
                # adds don't serialize on one engine
                if oi % 2 == 0:
                    nc.vector.tensor_tensor(
                        o_halves[oi // 4][:, oi % 4, :], ps,
                        bo_sb[:, oi:oi + 1].to_broadcast([P, TQ]), OP.add)
                else:
                    nc.scalar.add(o_halves[oi // 4][:, oi % 4, :], ps,
                                  bo_sb[:, oi:oi + 1])

            for oi in range(6):
                nc.tensor.matmul(ogrp[oi],
                                 wo_sb[:, GH - 1, oi * P:(oi + 1) * P],
                                 ut_sb[:, GH - 1, :], start=False, stop=True)
                o_emit(oi, ogrp[oi])
            for oi in (6, 7):
                ps = ppool.tile([P, 2, TQ], f32, name="o_ps",
                                tag="big")[:, 0, :]
                for hc in range(GH):
                    nc.tensor.matmul(ps[:],
                                     wo_sb[:, hc, oi * P:(oi + 1) * P],
                                     ut_sb[:, hc, :],
                                     start=(hc == 0), stop=(hc == GH - 1))
                o_emit(oi, ps[:])
            for pair in range(4):
                nc.sync.dma_start(
                    out.ap()[:, 2 * pair:2 * pair + 2, :],
                    o_halves[pair // 2][:, 2 * (pair % 2):2 * (pair % 2) + 2, :])

    nc.finalize()
    return nc


def _shard_inputs(inputs_q, inputs_kv, attention_mask, Wq, bq, Wk, bk, Wv, bv,
                  Wo, bo):
    bf16 = ml_dtypes.bfloat16
    f32 = np.float32

    def ptile(a2d, inner):
        """[R, C] row-major -> [P, R//P, C] partition-major, contiguous."""
        r, c = a2d.shape
        return np.ascontiguousarray(
            a2d.reshape(r // P, P, c).transpose(1, 0, 2)).astype(inner)

    in_maps = []
    xqT = [ptile(inputs_q[b].T, bf16) for b in range(B)]          # [P,KC,TQ]
    xkvT = [ptile(inputs_kv[b].T, bf16)                           # [P,NTC,KC,512]
            .reshape(P, KC, NTC, 512).transpose(0, 2, 1, 3).copy()
            for b in range(B)]
    maskT = [ptile(attention_mask[b].T.astype(np.float32), bf16)  # [P,NTB,TQ]
             for b in range(B)]
    for c in range(NCORES):
        b, g = c % B, c // B  # pair = (b, b+4)
        sl = slice(g * GD, (g + 1) * GD)
        in_maps.append({
            "xqT": xqT[b],
            "xkvT": xkvT[b],
            "maskT": maskT[b],
            "Wq": ptile(np.ascontiguousarray(Wq[:, sl]), bf16),
            "Wk": ptile(np.ascontiguousarray(Wk[:, sl]), bf16),
            "Wv": ptile(np.ascontiguousarray(Wv[:, sl]), bf16),
            "Wo": ptile(np.ascontiguousarray(Wo[sl, :]), bf16),
            "bq": np.ascontiguousarray(bq[sl]).astype(f32),
            "bk": np.ascontiguousarray(bk[sl]).astype(f32),
            "bv": np.ascontiguousarray(bv[sl]).astype(f32),
            "bo": (bo.astype(f32) if g == 0 else np.zeros(D, f32)),
        })
    return in_maps


def kernel(_trace=False, **inputs):
    global _CACHED_NC
    from concourse import bass_utils

    arrs = {k: np.asarray(v) for k, v in inputs.items()}
    in_maps = _shard_inputs(**arrs)

    if _CACHED_NC is None:
        _CACHED_NC = _build_nc()

    res = bass_utils.run_bass_kernel_spmd(
        _CACHED_NC, in_maps, core_ids=list(range(NCORES)), trace=_trace)

    full = np.empty((B, TQ, D), np.float32)
    for b in range(B):
        # pair (b, b+4): sum the two head-group partial outputs on the
        # host (the unshard step for a sum-sharded output)
        outT = (res.results[b]["out"].astype(np.float32)
                + res.results[b + 4]["out"].astype(np.float32))
        full[b] = outT.transpose(1, 0, 2).reshape(D, TQ).T
    if _trace:
        return full, res
    return full


# revision 40
# speedup vs baseline: 1.3110x; 1.0189x over previous
"""Distributed attention kernel for one TRN2 chip (8 NeuronCores).

Problem: multi-head cross-attention
  B=4, TQ=512, TKV=4096, D=1024, H=8 heads (head_dim=128)

Sharding (data-parallel x tensor-parallel, per the hint):
  core c in 0..7 -> (batch b = c % 4, head-group g = c // 4)
  Each core computes heads [4g, 4g+4) for its batch: Wq/Wk/Wv column
  shards, Wo row shard.  Each core writes its full partial out^T; the
  host sums the (c, c+4) pair during the gather (the unshard step for a
  sum-sharded output), so no on-device collective / rendezvous tail.

Device layout (per core; everything transposed so no on-device
transposes are needed - the host passes x^T and mask^T):
  Q^T[dh, t]  = Wq_g^T x_q^T          (4 head-blocks x 8 k-chunks)
  K^T[dh, T]  = Wk_g^T x_kv^T
  V[T, dh]    = x_kv Wv_g             (from x_kv^T chunks as lhsT)
  S^T[T, t]   = K^T_h(block)^T Q^T_h  per head, 32 T-blocks
  P^T         = exp(S^T/sqrt(128)) * mask^T   (no max-subtraction needed:
                scores are O(1) so exp cannot overflow/underflow)
  U^T[dh, t] += V_h(block)^T P^T      accumulated over T-blocks in PSUM
  den        += ones^T P^T            per-block M=1 matmuls into psum
                row 0 (softmax denominators for all t at once)
  U^T *= 1/(den+tiny)                 approx-reciprocal; rows with an
                all-false mask give U = 0 exactly, matching the
                reference's post-softmax wipe
  out^T[o, t] = Wo_g^T U^T (+ bo on group 0 only), DMA out per pair.

Attention loop is software-pipelined: exp+mask-mult for step ds+2
issue right behind that step's S matmuls, so ACT/DVE run a full step
ahead and the PE's semaphore waits are pre-satisfied (LDWEIGHTS
prefetch hides behind streaming).  Per-head finalize (broadcast,
reciprocal quarters, normalize halves) is dripped one op per step into
the next head so it never blocks an engine FIFO; the last head's
finalize overlaps the first 18 output-projection matmuls.

Matmul inputs are bf16 (PE 4x faster than fp32); PSUM accumulation,
softmax denominators and reciprocal stay fp32.
"""

import sys

if "/opt/trn_rl_repo" not in sys.path:
    sys.path.insert(0, "/opt/trn_rl_repo")

import numpy as np
import ml_dtypes
from contextlib import ExitStack

B, TQ, TKV, D, H = 4, 512, 4096, 1024, 8
HD = D // H            # 128 head dim
NCORES = 8
GH = H // 2            # heads per core = 4
GD = GH * HD           # 512 cols per head-group
P = 128
KC = D // P            # 8 contraction chunks
NTB = TKV // P         # 32 T-blocks
NTC = TKV // 512       # 8 T-chunks (DMA granularity)
NOB = D // P           # 8 output o-blocks
SCALE = float(1.0 / np.sqrt(HD))

_CACHED_NC = None


def _build_nc():
    from concourse import mybir, bacc
    from concourse.tile import TileContext

    bf = mybir.dt.bfloat16
    f32 = mybir.dt.float32
    AF = mybir.ActivationFunctionType
    OP = mybir.AluOpType

    nc = bacc.Bacc("TRN2", target_bir_lowering=False, debug=False,
                   num_devices=NCORES)

    # All inputs are pre-tiled on the host into partition-major layouts
    # so every DMA is 128 contiguous multi-KB descriptors.
    xqT = nc.dram_tensor("xqT", [P, KC, TQ], bf, kind="ExternalInput")
    xkvT = nc.dram_tensor("xkvT", [P, NTC, KC, 512], bf, kind="ExternalInput")
    maskT = nc.dram_tensor("maskT", [P, NTB, TQ], bf, kind="ExternalInput")
    Wq = nc.dram_tensor("Wq", [P, KC, GD], bf, kind="ExternalInput")
    Wk = nc.dram_tensor("Wk", [P, KC, GD], bf, kind="ExternalInput")
    Wv = nc.dram_tensor("Wv", [P, KC, GD], bf, kind="ExternalInput")
    Wo = nc.dram_tensor("Wo", [P, GH, D], bf, kind="ExternalInput")
    bq = nc.dram_tensor("bq", [GD], f32, kind="ExternalInput")
    bk = nc.dram_tensor("bk", [GD], f32, kind="ExternalInput")
    bv = nc.dram_tensor("bv", [GD], f32, kind="ExternalInput")
    bo = nc.dram_tensor("bo", [D], f32, kind="ExternalInput")
    out = nc.dram_tensor("out", [P, NOB, TQ], bf, kind="ExternalOutput")

    with TileContext(nc) as tc:
        with ExitStack() as ctx:
            persist = ctx.enter_context(tc.tile_pool(name="persist", bufs=1))
            kvchunk = ctx.enter_context(tc.tile_pool(name="kvchunk", bufs=3))
            work = ctx.enter_context(tc.tile_pool(name="work", bufs=3))
            outp = ctx.enter_context(tc.tile_pool(name="outp", bufs=2))
            # PSUM budget (8 banks): ppool 2x[P,2,TQ] = 4, upool 2x[P,TQ]
            # = 2, dpool 2x[P,TQ] = 2.
            ppool = ctx.enter_context(
                tc.tile_pool(name="ppool", bufs=2, space="PSUM"))
            upool = ctx.enter_context(
                tc.tile_pool(name="upool", bufs=2, space="PSUM"))
            dpool = ctx.enter_context(
                tc.tile_pool(name="dpool", bufs=2, space="PSUM"))

            # ---- constants / weights / biases -------------------------
            # Wq+xq first (whole tensors: 8KB-per-partition descriptors)
            # so the Q projection starts ~6us in, then Wk/kv0/Wv/kv1;
            # mask/Wo are only needed later.
            wq_sb = persist.tile([P, KC, GD], bf)
            xq_sb = persist.tile([P, KC, TQ], bf)
            for q in range(4):
                nc.sync.dma_start(wq_sb[:, 2 * q:2 * q + 2, :],
                                  Wq.ap()[:, 2 * q:2 * q + 2, :])
                nc.sync.dma_start(xq_sb[:, 2 * q:2 * q + 2, :],
                                  xqT.ap()[:, 2 * q:2 * q + 2, :])

            bq_sb = persist.tile([P, GH], f32)
            bk_sb = persist.tile([P, GH], f32)
            nc.sync.dma_start(bq_sb[:], bq.ap().rearrange("(h p) -> p h", p=P))
            nc.sync.dma_start(bk_sb[:], bk.ap().rearrange("(h p) -> p h", p=P))
            bv_row = persist.tile([1, GD], f32)
            nc.sync.dma_start(bv_row[:], bv.ap().unsqueeze(0))
            bv_rep = persist.tile([P, GD], f32)
            nc.gpsimd.partition_broadcast(bv_rep[:], bv_row[:])

            ones_bf = persist.tile([P, 1], bf)
            nc.vector.memset(ones_bf[:], 1.0)

            wk_sb = persist.tile([P, KC, GD], bf)
            wv_sb = persist.tile([P, KC, GD], bf)
            kv_tiles = {}

            def load_kv_chunk(tcknk, split=1):
                # chunks 0-1 are needed ~25us in but a 1MB dma_start
                # lands on a single queue (~6us + queue-init); split the
                # early ones across queues
                t = kvchunk.tile([P, KC, 512], bf, name="xkv_t", tag="xkv")
                n = KC // split
                for piece in range(split):
                    nc.sync.dma_start(
                        t[:, piece * n:(piece + 1) * n, :],
                        xkvT.ap()[:, tcknk, piece * n:(piece + 1) * n, :])
                kv_tiles[tcknk] = t

            for q in range(4):
                nc.sync.dma_start(wk_sb[:, 2 * q:2 * q + 2, :],
                                  Wk.ap()[:, 2 * q:2 * q + 2, :])
            load_kv_chunk(0, split=4)
            for q in range(4):
                nc.sync.dma_start(wv_sb[:, 2 * q:2 * q + 2, :],
                                  Wv.ap()[:, 2 * q:2 * q + 2, :])
            load_kv_chunk(1, split=4)

            # ---- Q^T = Wq_g^T x_q^T  (+bq) ----------------------------
            qt_sb = persist.tile([P, GH, TQ], bf)
            for db in range(GH):
                ps = ppool.tile([P, 2, TQ], f32, name="proj_ps",
                                tag="big")[:, 0, :]
                for kc in range(KC):
                    nc.tensor.matmul(ps[:], wq_sb[:, kc, db * P:(db + 1) * P],
                                     xq_sb[:, kc, :],
                                     start=(kc == 0), stop=(kc == KC - 1))
                nc.vector.tensor_tensor(
                    qt_sb[:, db, :], ps[:],
                    bq_sb[:, db:db + 1].to_broadcast([P, TQ]), OP.add)

            # ---- K^T and V over T-chunks ------------------------------
            kt_sb = persist.tile([P, GH, TKV], bf)
            v_sb = persist.tile([P, NTB, GD], bf)
            mask_sb = persist.tile([P, NTB, TQ], bf)
            bo_sb = persist.tile([P, NOB], f32)
            wo_sb = persist.tile([P, GH, D], bf)
            for tcknk in range(NTC):
                if tcknk + 2 < NTC:
                    load_kv_chunk(tcknk + 2)
                xkv_t = kv_tiles.pop(tcknk)
                if tcknk == 3:
                    # the 4MB mask saturates HBM if issued early; issue
                    # it here - past the urgent wk/wv/chunk loads, still
                    # ~70us before attention needs it
                    nc.sync.dma_start(mask_sb[:], maskT.ap())
                    nc.sync.dma_start(wo_sb[:], Wo.ap())
                    nc.sync.dma_start(
                        bo_sb[:], bo.ap().rearrange("(ob p) -> p ob", p=P))
                for db in range(GH):
                    ps = ppool.tile([P, 2, TQ], f32, name="proj_ps",
                                    tag="big")[:, 0, :]
                    for kc in range(KC):
                        nc.tensor.matmul(ps[:], wk_sb[:, kc, db * P:(db + 1) * P],
                                         xkv_t[:, kc, :],
                                         start=(kc == 0), stop=(kc == KC - 1))
                    nc.vector.tensor_tensor(
                        kt_sb[:, db, tcknk * 512:(tcknk + 1) * 512], ps[:],
                        bk_sb[:, db:db + 1].to_broadcast([P, 512]), OP.add)
                for tb in range(4):
                    ps = ppool.tile([P, 2, TQ], f32, name="proj_ps",
                                    tag="big")[:, 0, :]
                    for kc in range(KC):
                        nc.tensor.matmul(ps[:],
                                         xkv_t[:, kc, tb * P:(tb + 1) * P],
                                         wv_sb[:, kc, :],
                                         start=(kc == 0), stop=(kc == KC - 1))
                    nc.vector.tensor_tensor(
                        v_sb[:, tcknk * 4 + tb, :], ps[:], bv_rep[:], OP.add)

            # ---- attention, software-pipelined double-step loop -------
            # Two T-blocks per step: two S-matmuls fill the two banks of
            # one [P, 2, TQ] psum tile, then ONE wide exp + mask-mult.
            # exp/mult run one step AHEAD of the U matmuls that consume
            # them; S prefetch runs two ahead.  Tensor order per step is
            # U (deps long ready), den ones-matmuls, then the next S
            # pair.  With p_t ready a full step early the PE's LDWEIGHTS
            # prefetch is never semaphore-blocked.
            ut_sb = persist.tile([P, GH, TQ], bf)
            NDS = GH * NTB // 2
            s_tiles, p_tiles, p01_tiles = {}, {}, {}
            u_tiles = [None] * GH
            den_tiles = [None] * GH
            fin_pend = None

            def s2_mm(ds):
                t2 = ppool.tile([P, 2, TQ], f32, name="s2_ps", tag="big")
                for k in range(2):
                    h, j = divmod(ds * 2 + k, NTB)
                    nc.tensor.matmul(t2[:, k, :],
                                     kt_sb[:, h, j * P:(j + 1) * P],
                                     qt_sb[:, h, :], start=True, stop=True)
                return t2

            def exp_mult(ds):
                h, j0 = divmod(ds * 2, NTB)
                t2 = s_tiles.pop(ds)
                praw = work.tile([P, 2, TQ], bf, tag="praw", bufs=2)
                nc.scalar.activation(praw[:], t2[:], AF.Exp, scale=SCALE)
                p_t = work.tile([P, 2, TQ], bf, tag="p_t", bufs=4)
                nc.vector.tensor_tensor(p_t[:], praw[:],
                                        mask_sb[:, j0:j0 + 2, :], OP.mult)
                p_tiles[ds] = p_t
                # pair-sum the two P blocks on DVE (bf16 2x rate, and it
                # has ~350ns/step of slack) so den needs ONE ones-matmul
                # per step instead of two on the pacing tensor engine
                p01 = work.tile([P, TQ], bf, tag="p01", bufs=3)
                nc.vector.tensor_tensor(p01[:], p_t[:, 0, :], p_t[:, 1, :],
                                        OP.add)
                p01_tiles[ds] = p01

            def fin_chain(h, drow):
                # 1/den broadcast + normalize.  Returns a list of small
                # thunks so the DVE work can be dripped one op per step
                # (a single fat op at a head boundary blocks the DVE
                # FIFO and stalls the U matmuls behind it).
                rep = work.tile([P, TQ], f32, tag="rep", bufs=2)
                rcp = work.tile([P, TQ], f32, tag="rcp", bufs=2)
                u_ps = u_tiles[h]
                ops = [lambda: nc.gpsimd.partition_broadcast(rep[:], drow[:])]
                for q in range(4):
                    sl = slice(q * TQ // 4, (q + 1) * TQ // 4)
                    ops.append(lambda sl=sl: nc.vector.reciprocal_approx_fast(
                        rcp[:, sl], rep[:, sl]))
                for g in range(2):
                    sl = slice(g * TQ // 2, (g + 1) * TQ // 2)
                    ops.append(lambda sl=sl: nc.vector.tensor_tensor(
                        ut_sb[:, h, sl], u_ps[:, sl], rcp[:, sl], OP.mult))
                return ops

            # prologue
            s_tiles[0] = s2_mm(0)
            s_tiles[1] = s2_mm(1)
            exp_mult(0)
            exp_mult(1)
            fin_ops = []

            for ds in range(NDS):
                h, j0 = divmod(ds * 2, NTB)
                if j0 == 0:
                    u_tiles[h] = upool.tile([P, TQ], f32, name="u_ps",
                                            tag="u_ps")
                    den_tiles[h] = dpool.tile([P, TQ], f32, name="den_ps",
                                              tag="den_ps")
                if fin_ops:
                    fin_ops.pop(0)()  # drip one finalize op per step
                p_t = p_tiles.pop(ds)
                # Grouped same-type pairs (U,U,den,den,S,S) measure
                # ~150ns/step faster than any interleaving of U/den/S.
                for k in range(2):
                    j = j0 + k
                    nc.tensor.matmul(u_tiles[h][:],
                                     v_sb[:, j, h * P:(h + 1) * P],
                                     p_t[:, k, :],
                                     start=(j == 0), stop=(j == NTB - 1))
                nc.tensor.matmul(den_tiles[h][0:1, :], ones_bf[:],
                                 p01_tiles.pop(ds)[:],
                                 start=(j0 == 0), stop=(j0 == NTB - 2))
                if ds + 2 < NDS:
                    s_tiles[ds + 2] = s2_mm(ds + 2)
                    # exp/mask-mult two steps ahead, right behind their
                    # S matmuls: ACT and DVE get a full step of slack
                    exp_mult(ds + 2)
                if j0 == NTB - 2:
                    # den row -> sbuf on the idle ACT engine; +1e-30 is
                    # the all-masked-row guard, folded in for free
                    drow = work.tile([1, TQ], f32, tag="drow", bufs=2)
                    nc.scalar.activation(drow[:], den_tiles[h][0:1, :],
                                         AF.Copy, bias=1e-30)
                    fin_pend = (h, drow)
                if fin_pend is not None and j0 == 0 and ds > 0:
                    fin_ops = fin_chain(*fin_pend)
                    fin_pend = None

            # ---- out^T = Wo_g^T U^T (+bo), direct DMA out -------------
            # Partial (head-group) output; the host sums the pair.
            # Phase 1 accumulates heads 0..2 for 6 o-blocks across every
            # free psum bank while head 3's finalize chain runs on
            # gpsimd/DVE; phase 2 adds head 3 and the last 2 o-blocks.
            o_halves = [outp.tile([P, NOB // 2, TQ], bf, name="o_half",
                                  tag="o_half") for _ in range(2)]
            ppA = ppool.tile([P, 2, TQ], f32, name="o_ps", tag="big")
            ppB = ppool.tile([P, 2, TQ], f32, name="o_ps", tag="big")
            ogrp = [ppA[:, 0, :], ppA[:, 1, :], ppB[:, 0, :], ppB[:, 1, :],
                    upool.tile([P, TQ], f32, name="o_ps", tag="u_ps")[:],
                    dpool.tile([P, TQ], f32, name="o_ps", tag="den_ps")[:]]
            for oi in range(6):
                for hc in range(GH - 1):
                    nc.tensor.matmul(ogrp[oi],
                                     wo_sb[:, hc, oi * P:(oi + 1) * P],
                                     ut_sb[:, hc, :],
                                     start=(hc == 0), stop=False)
            # head 3 finalize; overlaps the phase-1 matmuls above
            for op in fin_chain(*fin_pend):
                op()
            fin_pend = None

            def o_emit(oi, ps):
                # alternate the psum->sbuf bias-add between DVE and ACT
                # (bo is per-partition in the out^T layout) so the tail
                # adds don't serialize on one engine
                if oi % 2 == 0:
                    nc.vector.tensor_tensor(
                        o_halves[oi // 4][:, oi % 4, :], ps,
                        bo_sb[:, oi:oi + 1].to_broadcast([P, TQ]), OP.add)
                else:
                    nc.scalar.add(o_halves[oi // 4][:, oi % 4, :], ps,
                                  bo_sb[:, oi:oi + 1])

            for oi in range(6):
                nc.tensor.matmul(ogrp[oi],
                                 wo_sb[:, GH - 1, oi * P:(oi + 1) * P],
                                 ut_sb[:, GH - 1, :], start=False, stop=True)
                o_emit(oi, ogrp[oi])
            for oi in (6, 7):
                ps = ppool.tile([P, 2, TQ], f32, name="o_ps",
                                tag="big")[:, 0, :]
                for hc in range(GH):
                    nc.tensor.matmul(ps[:],
                                     wo_sb[:, hc, oi * P:(oi + 1) * P],
                                     ut_sb[:, hc, :],
                                     start=(hc == 0), stop=(hc == GH - 1))
                o_emit(oi, ps[:])
            for pair in range(4):
                nc.sync.dma_start(
                    out.ap()[:, 2 * pair:2 * pair + 2, :],
                    o_halves[pair // 2][:, 2 * (pair % 2):2 * (pair % 2) + 2, :])

    nc.finalize()
    return nc


def _shard_inputs(inputs_q, inputs_kv, attention_mask, Wq, bq, Wk, bk, Wv, bv,
                  Wo, bo):
    bf16 = ml_dtypes.bfloat16
    f32 = np.float32

    def ptile(a2d, inner):
        """[R, C] row-major -> [P, R//P, C] partition-major, contiguous."""
        r, c = a2d.shape
        return np.ascontiguousarray(
            a2d.reshape(r // P, P, c).transpose(1, 0, 2)).astype(inner)

    in_maps = []
    xqT = [ptile(inputs_q[b].T, bf16) for b in range(B)]          # [P,KC,TQ]
    xkvT = [ptile(inputs_kv[b].T, bf16)                           # [P,NTC,KC,512]
            .reshape(P, KC, NTC, 512).transpose(0, 2, 1, 3).copy()
            for b in range(B)]
    maskT = [ptile(attention_mask[b].T.astype(np.float32), bf16)  # [P,NTB,TQ]
             for b in range(B)]
    for c in range(NCORES):
        b, g = c % B, c // B  # pair = (b, b+4)
        sl = slice(g * GD, (g + 1) * GD)
        in_maps.append({
            "xqT": xqT[b],
            "xkvT": xkvT[b],
            "maskT": maskT[b],
            "Wq": ptile(np.ascontiguousarray(Wq[:, sl]), bf16),
            "Wk": ptile(np.ascontiguousarray(Wk[:, sl]), bf16),
            "Wv": ptile(np.ascontiguousarray(Wv[:, sl]), bf16),
            "Wo": ptile(np.ascontiguousarray(Wo[sl, :]), bf16),
            "bq": np.ascontiguousarray(bq[sl]).astype(f32),
            "bk": np.ascontiguousarray(bk[sl]).astype(f32),
            "bv": np.ascontiguousarray(bv[sl]).astype(f32),
            "bo": (bo.astype(f32) if g == 0 else np.zeros(D, f32)),
        })
    return in_maps


def kernel(_trace=False, **inputs):
    global _CACHED_NC
    from concourse import bass_utils

    arrs = {k: np.asarray(v) for k, v in inputs.items()}
    in_maps = _shard_inputs(**arrs)

    if _CACHED_NC is None:
        _CACHED_NC = _build_nc()

    res = bass_utils.run_bass_kernel_spmd(
        _CACHED_NC, in_maps, core_ids=list(range(NCORES)), trace=_trace)

    full = np.empty((B, TQ, D), np.float32)
    for b in range(B):
        # pair (b, b+4): sum the two head-group partial outputs on the
        # host (the unshard step for a sum-sharded output)
        outT = (res.results[b]["out"].astype(np.float32)
                + res.results[b + 4]["out"].astype(np.float32))
        full[b] = outT.transpose(1, 0, 2).reshape(D, TQ).T
    if _trace:
        return full, res
    return full


# revision 41
# speedup vs baseline: 1.3145x; 1.0026x over previous
"""Distributed attention kernel for one TRN2 chip (8 NeuronCores).

Problem: multi-head cross-attention
  B=4, TQ=512, TKV=4096, D=1024, H=8 heads (head_dim=128)

Sharding (data-parallel x tensor-parallel, per the hint):
  core c in 0..7 -> (batch b = c % 4, head-group g = c // 4)
  Each core computes heads [4g, 4g+4) for its batch: Wq/Wk/Wv column
  shards, Wo row shard.  Each core writes its full partial out^T; the
  host sums the (c, c+4) pair during the gather (the unshard step for a
  sum-sharded output), so no on-device collective / rendezvous tail.

Device layout (per core; everything transposed so no on-device
transposes are needed - the host passes x^T and mask^T):
  Q^T[dh, t]  = Wq_g^T x_q^T          (4 head-blocks x 8 k-chunks)
  K^T[dh, T]  = Wk_g^T x_kv^T
  V[T, dh]    = x_kv Wv_g             (from x_kv^T chunks as lhsT)
  S^T[T, t]   = K^T_h(block)^T Q^T_h  per head, 32 T-blocks
  P^T         = exp(S^T/sqrt(128)) * mask^T   (no max-subtraction needed:
                scores are O(1) so exp cannot overflow/underflow)
  U^T[dh, t] += V_h(block)^T P^T      accumulated over T-blocks in PSUM
  den        += ones^T P^T            per-block M=1 matmuls into psum
                row 0 (softmax denominators for all t at once)
  U^T *= 1/(den+tiny)                 approx-reciprocal; rows with an
                all-false mask give U = 0 exactly, matching the
                reference's post-softmax wipe
  out^T[o, t] = Wo_g^T U^T (+ bo on group 0 only), DMA out per pair.

Attention loop is software-pipelined: exp+mask-mult for step ds+2
issue right behind that step's S matmuls, so ACT/DVE run a full step
ahead and the PE's semaphore waits are pre-satisfied (LDWEIGHTS
prefetch hides behind streaming).  Per-head finalize (broadcast,
reciprocal quarters, normalize halves) is dripped one op per step into
the next head so it never blocks an engine FIFO; the last head's
finalize overlaps the first 18 output-projection matmuls.

Matmul inputs are bf16 (PE 4x faster than fp32); PSUM accumulation,
softmax denominators and reciprocal stay fp32.
"""

import sys

if "/opt/trn_rl_repo" not in sys.path:
    sys.path.insert(0, "/opt/trn_rl_repo")

import numpy as np
import ml_dtypes
from contextlib import ExitStack

B, TQ, TKV, D, H = 4, 512, 4096, 1024, 8
HD = D // H            # 128 head dim
NCORES = 8
GH = H // 2            # heads per core = 4
GD = GH * HD           # 512 cols per head-group
P = 128
KC = D // P            # 8 contraction chunks
NTB = TKV // P         # 32 T-blocks
NTC = TKV // 512       # 8 T-chunks (DMA granularity)
NOB = D // P           # 8 output o-blocks
SCALE = float(1.0 / np.sqrt(HD))

_CACHED_NC = None


def _build_nc():
    from concourse import mybir, bacc
    from concourse.tile import TileContext

    bf = mybir.dt.bfloat16
    f32 = mybir.dt.float32
    AF = mybir.ActivationFunctionType
    OP = mybir.AluOpType

    nc = bacc.Bacc("TRN2", target_bir_lowering=False, debug=False,
                   num_devices=NCORES)

    # All inputs are pre-tiled on the host into partition-major layouts
    # so every DMA is 128 contiguous multi-KB descriptors.
    xqT = nc.dram_tensor("xqT", [P, KC, TQ], bf, kind="ExternalInput")
    xkvT = nc.dram_tensor("xkvT", [P, NTC, KC, 512], bf, kind="ExternalInput")
    maskT = nc.dram_tensor("maskT", [P, NTB, TQ], bf, kind="ExternalInput")
    Wq = nc.dram_tensor("Wq", [P, KC, GD], bf, kind="ExternalInput")
    Wk = nc.dram_tensor("Wk", [P, KC, GD], bf, kind="ExternalInput")
    Wv = nc.dram_tensor("Wv", [P, KC, GD], bf, kind="ExternalInput")
    Wo = nc.dram_tensor("Wo", [P, GH, D], bf, kind="ExternalInput")
    bq = nc.dram_tensor("bq", [GD], f32, kind="ExternalInput")
    bk = nc.dram_tensor("bk", [GD], f32, kind="ExternalInput")
    bv = nc.dram_tensor("bv", [GD], f32, kind="ExternalInput")
    bo = nc.dram_tensor("bo", [D], f32, kind="ExternalInput")
    out = nc.dram_tensor("out", [P, NOB, TQ], bf, kind="ExternalOutput")

    with TileContext(nc) as tc:
        with ExitStack() as ctx:
            persist = ctx.enter_context(tc.tile_pool(name="persist", bufs=1))
            kvchunk = ctx.enter_context(tc.tile_pool(name="kvchunk", bufs=3))
            work = ctx.enter_context(tc.tile_pool(name="work", bufs=3))
            outp = ctx.enter_context(tc.tile_pool(name="outp", bufs=2))
            # PSUM budget (8 banks): ppool 2x[P,2,TQ] = 4, upool 2x[P,TQ]
            # = 2, dpool 2x[P,TQ] = 2.
            ppool = ctx.enter_context(
                tc.tile_pool(name="ppool", bufs=2, space="PSUM"))
            upool = ctx.enter_context(
                tc.tile_pool(name="upool", bufs=2, space="PSUM"))
            dpool = ctx.enter_context(
                tc.tile_pool(name="dpool", bufs=2, space="PSUM"))

            # ---- constants / weights / biases -------------------------
            # Wq+xq first (whole tensors: 8KB-per-partition descriptors)
            # so the Q projection starts ~6us in, then Wk/kv0/Wv/kv1;
            # mask/Wo are only needed later.
            wq_sb = persist.tile([P, KC, GD], bf)
            xq_sb = persist.tile([P, KC, TQ], bf)
            for a, b in ((0, 1), (1, 3), (3, 5), (5, 8)):
                nc.sync.dma_start(wq_sb[:, a:b, :], Wq.ap()[:, a:b, :])
                nc.sync.dma_start(xq_sb[:, a:b, :], xqT.ap()[:, a:b, :])

            bq_sb = persist.tile([P, GH], f32)
            bk_sb = persist.tile([P, GH], f32)
            nc.sync.dma_start(bq_sb[:], bq.ap().rearrange("(h p) -> p h", p=P))
            nc.sync.dma_start(bk_sb[:], bk.ap().rearrange("(h p) -> p h", p=P))
            bv_row = persist.tile([1, GD], f32)
            nc.sync.dma_start(bv_row[:], bv.ap().unsqueeze(0))
            bv_rep = persist.tile([P, GD], f32)
            nc.gpsimd.partition_broadcast(bv_rep[:], bv_row[:])

            ones_bf = persist.tile([P, 1], bf)
            nc.vector.memset(ones_bf[:], 1.0)

            wk_sb = persist.tile([P, KC, GD], bf)
            wv_sb = persist.tile([P, KC, GD], bf)
            kv_tiles = {}

            def load_kv_chunk(tcknk, split=1):
                # chunks 0-1 are needed ~25us in but a 1MB dma_start
                # lands on a single queue (~6us + queue-init); split the
                # early ones across queues
                t = kvchunk.tile([P, KC, 512], bf, name="xkv_t", tag="xkv")
                n = KC // split
                for piece in range(split):
                    nc.sync.dma_start(
                        t[:, piece * n:(piece + 1) * n, :],
                        xkvT.ap()[:, tcknk, piece * n:(piece + 1) * n, :])
                kv_tiles[tcknk] = t

            for q in range(4):
                nc.sync.dma_start(wk_sb[:, 2 * q:2 * q + 2, :],
                                  Wk.ap()[:, 2 * q:2 * q + 2, :])
            load_kv_chunk(0, split=4)
            for q in range(4):
                nc.sync.dma_start(wv_sb[:, 2 * q:2 * q + 2, :],
                                  Wv.ap()[:, 2 * q:2 * q + 2, :])
            load_kv_chunk(1, split=4)

            # ---- Q^T = Wq_g^T x_q^T  (+bq) ----------------------------
            qt_sb = persist.tile([P, GH, TQ], bf)
            for db in range(GH):
                ps = ppool.tile([P, 2, TQ], f32, name="proj_ps",
                                tag="big")[:, 0, :]
                for kc in range(KC):
                    nc.tensor.matmul(ps[:], wq_sb[:, kc, db * P:(db + 1) * P],
                                     xq_sb[:, kc, :],
                                     start=(kc == 0), stop=(kc == KC - 1))
                nc.vector.tensor_tensor(
                    qt_sb[:, db, :], ps[:],
                    bq_sb[:, db:db + 1].to_broadcast([P, TQ]), OP.add)

            # ---- K^T and V over T-chunks ------------------------------
            kt_sb = persist.tile([P, GH, TKV], bf)
            v_sb = persist.tile([P, NTB, GD], bf)
            mask_sb = persist.tile([P, NTB, TQ], bf)
            bo_sb = persist.tile([P, NOB], f32)
            wo_sb = persist.tile([P, GH, D], bf)
            for tcknk in range(NTC):
                if tcknk + 2 < NTC:
                    load_kv_chunk(tcknk + 2)
                xkv_t = kv_tiles.pop(tcknk)
                if tcknk == 3:
                    # the 4MB mask saturates HBM if issued early; issue
                    # it here - past the urgent wk/wv/chunk loads, still
                    # ~70us before attention needs it
                    nc.sync.dma_start(mask_sb[:], maskT.ap())
                    nc.sync.dma_start(wo_sb[:], Wo.ap())
                    nc.sync.dma_start(
                        bo_sb[:], bo.ap().rearrange("(ob p) -> p ob", p=P))
                for db in range(GH):
                    ps = ppool.tile([P, 2, TQ], f32, name="proj_ps",
                                    tag="big")[:, 0, :]
                    for kc in range(KC):
                        nc.tensor.matmul(ps[:], wk_sb[:, kc, db * P:(db + 1) * P],
                                         xkv_t[:, kc, :],
                                         start=(kc == 0), stop=(kc == KC - 1))
                    nc.vector.tensor_tensor(
                        kt_sb[:, db, tcknk * 512:(tcknk + 1) * 512], ps[:],
                        bk_sb[:, db:db + 1].to_broadcast([P, 512]), OP.add)
                for tb in range(4):
                    ps = ppool.tile([P, 2, TQ], f32, name="proj_ps",
                                    tag="big")[:, 0, :]
                    for kc in range(KC):
                        nc.tensor.matmul(ps[:],
                                         xkv_t[:, kc, tb * P:(tb + 1) * P],
                                         wv_sb[:, kc, :],
                                         start=(kc == 0), stop=(kc == KC - 1))
                    nc.vector.tensor_tensor(
                        v_sb[:, tcknk * 4 + tb, :], ps[:], bv_rep[:], OP.add)

            # ---- attention, software-pipelined double-step loop -------
            # Two T-blocks per step: two S-matmuls fill the two banks of
            # one [P, 2, TQ] psum tile, then ONE wide exp + mask-mult.
            # exp/mult run one step AHEAD of the U matmuls that consume
            # them; S prefetch runs two ahead.  Tensor order per step is
            # U (deps long ready), den ones-matmuls, then the next S
            # pair.  With p_t ready a full step early the PE's LDWEIGHTS
            # prefetch is never semaphore-blocked.
            ut_sb = persist.tile([P, GH, TQ], bf)
            NDS = GH * NTB // 2
            s_tiles, p_tiles, p01_tiles = {}, {}, {}
            u_tiles = [None] * GH
            den_tiles = [None] * GH
            fin_pend = None

            def s2_mm(ds):
                t2 = ppool.tile([P, 2, TQ], f32, name="s2_ps", tag="big")
                for k in range(2):
                    h, j = divmod(ds * 2 + k, NTB)
                    nc.tensor.matmul(t2[:, k, :],
                                     kt_sb[:, h, j * P:(j + 1) * P],
                                     qt_sb[:, h, :], start=True, stop=True)
                return t2

            def exp_mult(ds):
                h, j0 = divmod(ds * 2, NTB)
                t2 = s_tiles.pop(ds)
                praw = work.tile([P, 2, TQ], bf, tag="praw", bufs=2)
                nc.scalar.activation(praw[:], t2[:], AF.Exp, scale=SCALE)
                p_t = work.tile([P, 2, TQ], bf, tag="p_t", bufs=4)
                nc.vector.tensor_tensor(p_t[:], praw[:],
                                        mask_sb[:, j0:j0 + 2, :], OP.mult)
                p_tiles[ds] = p_t
                # pair-sum the two P blocks on DVE (bf16 2x rate, and it
                # has ~350ns/step of slack) so den needs ONE ones-matmul
                # per step instead of two on the pacing tensor engine
                p01 = work.tile([P, TQ], bf, tag="p01", bufs=3)
                nc.vector.tensor_tensor(p01[:], p_t[:, 0, :], p_t[:, 1, :],
                                        OP.add)
                p01_tiles[ds] = p01

            def fin_chain(h, drow):
                # 1/den broadcast + normalize.  Returns a list of small
                # thunks so the DVE work can be dripped one op per step
                # (a single fat op at a head boundary blocks the DVE
                # FIFO and stalls the U matmuls behind it).
                rep = work.tile([P, TQ], f32, tag="rep", bufs=2)
                rcp = work.tile([P, TQ], f32, tag="rcp", bufs=2)
                u_ps = u_tiles[h]
                ops = [lambda: nc.gpsimd.partition_broadcast(rep[:], drow[:])]
                for q in range(4):
                    sl = slice(q * TQ // 4, (q + 1) * TQ // 4)
                    ops.append(lambda sl=sl: nc.vector.reciprocal_approx_fast(
                        rcp[:, sl], rep[:, sl]))
                for g in range(2):
                    sl = slice(g * TQ // 2, (g + 1) * TQ // 2)
                    ops.append(lambda sl=sl: nc.vector.tensor_tensor(
                        ut_sb[:, h, sl], u_ps[:, sl], rcp[:, sl], OP.mult))
                return ops

            # prologue
            s_tiles[0] = s2_mm(0)
            s_tiles[1] = s2_mm(1)
            exp_mult(0)
            exp_mult(1)
            fin_ops = []

            for ds in range(NDS):
                h, j0 = divmod(ds * 2, NTB)
                if j0 == 0:
                    u_tiles[h] = upool.tile([P, TQ], f32, name="u_ps",
                                            tag="u_ps")
                    den_tiles[h] = dpool.tile([P, TQ], f32, name="den_ps",
                                              tag="den_ps")
                if fin_ops:
                    fin_ops.pop(0)()  # drip one finalize op per step
                p_t = p_tiles.pop(ds)
                # Grouped same-type pairs (U,U,den,den,S,S) measure
                # ~150ns/step faster than any interleaving of U/den/S.
                for k in range(2):
                    j = j0 + k
                    nc.tensor.matmul(u_tiles[h][:],
                                     v_sb[:, j, h * P:(h + 1) * P],
                                     p_t[:, k, :],
                                     start=(j == 0), stop=(j == NTB - 1))
                # den between the S pair: its 1-column weight load
                # is free, hiding S1's LDWEIGHTS behind den's streaming
                if ds + 2 < NDS:
                    s2 = ppool.tile([P, 2, TQ], f32, name="s2_ps",
                                    tag="big")
                    s_tiles[ds + 2] = s2
                    hs, js = divmod((ds + 2) * 2, NTB)
                    nc.tensor.matmul(s2[:, 0, :],
                                     kt_sb[:, hs, js * P:(js + 1) * P],
                                     qt_sb[:, hs, :], start=True, stop=True)
                nc.tensor.matmul(den_tiles[h][0:1, :], ones_bf[:],
                                 p01_tiles.pop(ds)[:],
                                 start=(j0 == 0), stop=(j0 == NTB - 2))
                if ds + 2 < NDS:
                    hs, js = divmod((ds + 2) * 2 + 1, NTB)
                    nc.tensor.matmul(s2[:, 1, :],
                                     kt_sb[:, hs, js * P:(js + 1) * P],
                                     qt_sb[:, hs, :], start=True, stop=True)
                    # exp/mask-mult two steps ahead, right behind their
                    # S matmuls: ACT and DVE get a full step of slack
                    exp_mult(ds + 2)
                if j0 == NTB - 2:
                    # den row -> sbuf on the idle ACT engine; +1e-30 is
                    # the all-masked-row guard, folded in for free
                    drow = work.tile([1, TQ], f32, tag="drow", bufs=2)
                    nc.scalar.activation(drow[:], den_tiles[h][0:1, :],
                                         AF.Copy, bias=1e-30)
                    fin_pend = (h, drow)
                if fin_pend is not None and j0 == 0 and ds > 0:
                    fin_ops = fin_chain(*fin_pend)
                    fin_pend = None

            # ---- out^T = Wo_g^T U^T (+bo), direct DMA out -------------
            # Partial (head-group) output; the host sums the pair.
            # Phase 1 accumulates heads 0..2 for 6 o-blocks across every
            # free psum bank while head 3's finalize chain runs on
            # gpsimd/DVE; phase 2 adds head 3 and the last 2 o-blocks.
            o_halves = [outp.tile([P, NOB // 2, TQ], bf, name="o_half",
                                  tag="o_half") for _ in range(2)]
            ppA = ppool.tile([P, 2, TQ], f32, name="o_ps", tag="big")
            ppB = ppool.tile([P, 2, TQ], f32, name="o_ps", tag="big")
            ogrp = [ppA[:, 0, :], ppA[:, 1, :], ppB[:, 0, :], ppB[:, 1, :],
                    upool.tile([P, TQ], f32, name="o_ps", tag="u_ps")[:],
                    dpool.tile([P, TQ], f32, name="o_ps", tag="den_ps")[:]]
            for oi in range(6):
                for hc in range(GH - 1):
                    nc.tensor.matmul(ogrp[oi],
                                     wo_sb[:, hc, oi * P:(oi + 1) * P],
                                     ut_sb[:, hc, :],
                                     start=(hc == 0), stop=False)
            # head 3 finalize; overlaps the phase-1 matmuls above
            for op in fin_chain(*fin_pend):
                op()
            fin_pend = None

            def o_emit(oi, ps):
                # alternate the psum->sbuf bias-add between DVE and ACT
                # (bo is per-partition in the out^T layout) so the tail
                # adds don't serialize on one engine
                if oi % 2 == 0:
                    nc.vector.tensor_tensor(
                        o_halves[oi // 4][:, oi % 4, :], ps,
                        bo_sb[:, oi:oi + 1].to_broadcast([P, TQ]), OP.add)
                else:
                    nc.scalar.add(o_halves[oi // 4][:, oi % 4, :], ps,
                                  bo_sb[:, oi:oi + 1])

            for oi in range(6):
                nc.tensor.matmul(ogrp[oi],
                                 wo_sb[:, GH - 1, oi * P:(oi + 1) * P],
                                 ut_sb[:, GH - 1, :], start=False, stop=True)
                o_emit(oi, ogrp[oi])
            for oi in (6, 7):
                ps = ppool.tile([P, 2, TQ], f32, name="o_ps",
                                tag="big")[:, 0, :]
                for hc in range(GH):
                    nc.tensor.matmul(ps[:],
                                     wo_sb[:, hc, oi * P:(oi + 1) * P],
                                     ut_sb[:, hc, :],
                                     start=(hc == 0), stop=(hc == GH - 1))
                o_emit(oi, ps[:])
            for pair in range(4):
                nc.sync.dma_start(
                    out.ap()[:, 2 * pair:2 * pair + 2, :],
                    o_halves[pair // 2][:, 2 * (pair % 2):2 * (pair % 2) + 2, :])

    nc.finalize()
    return nc


def _shard_inputs(inputs_q, inputs_kv, attention_mask, Wq, bq, Wk, bk, Wv, bv,
                  Wo, bo):
    bf16 = ml_dtypes.bfloat16
    f32 = np.float32

    def ptile(a2d, inner):
        """[R, C] row-major -> [P, R//P, C] partition-major, contiguous."""
        r, c = a2d.shape
        return np.ascontiguousarray(
            a2d.reshape(r // P, P, c).transpose(1, 0, 2)).astype(inner)

    in_maps = []
    xqT = [ptile(inputs_q[b].T, bf16) for b in range(B)]          # [P,KC,TQ]
    xkvT = [ptile(inputs_kv[b].T, bf16)                           # [P,NTC,KC,512]
            .reshape(P, KC, NTC, 512).transpose(0, 2, 1, 3).copy()
            for b in range(B)]
    maskT = [ptile(attention_mask[b].T.astype(np.float32), bf16)  # [P,NTB,TQ]
             for b in range(B)]
    for c in range(NCORES):
        b, g = c % B, c // B  # pair = (b, b+4)
        sl = slice(g * GD, (g + 1) * GD)
        in_maps.append({
            "xqT": xqT[b],
            "xkvT": xkvT[b],
            "maskT": maskT[b],
            "Wq": ptile(np.ascontiguousarray(Wq[:, sl]), bf16),
            "Wk": ptile(np.ascontiguousarray(Wk[:, sl]), bf16),
            "Wv": ptile(np.ascontiguousarray(Wv[:, sl]), bf16),
            "Wo": ptile(np.ascontiguousarray(Wo[sl, :]), bf16),
            "bq": np.ascontiguousarray(bq[sl]).astype(f32),
            "bk": np.ascontiguousarray(bk[sl]).astype(f32),
            "bv": np.ascontiguousarray(bv[sl]).astype(f32),
            "bo": (bo.astype(f32) if g == 0 else np.zeros(D, f32)),
        })
    return in_maps


def kernel(_trace=False, **inputs):
    global _CACHED_NC
    from concourse import bass_utils

    arrs = {k: np.asarray(v) for k, v in inputs.items()}
    in_maps = _shard_inputs(**arrs)

    if _CACHED_NC is None:
        _CACHED_NC = _build_nc()

    res = bass_utils.run_bass_kernel_spmd(
        _CACHED_NC, in_maps, core_ids=list(range(NCORES)), trace=_trace)

    full = np.empty((B, TQ, D), np.float32)
    for b in range(B):
        # pair (b, b+4): sum the two head-group partial outputs on the
        # host (the unshard step for a sum-sharded output)
        outT = (res.results[b]["out"].astype(np.float32)
                + res.results[b + 4]["out"].astype(np.float32))
        full[b] = outT.transpose(1, 0, 2).reshape(D, TQ).T
    if _trace:
        return full, res
    return full
